# revision 1
# baseline (speedup 1.0000x reference)
"""Causal self-attention TRN2 Bass kernel.

Sharding: 8 cores = 4 batches x 2 head-groups. Core c handles batch c//2 and
heads (c%2)*8 .. (c%2)*8+8 (of 16). Each core computes its heads' attention
and a partial output projection; the host sums the two partials per batch and
adds b_out.

All matmuls run in float32r (fp32 storage, reduced-mantissa multiplies at full
PE rate). Intermediates accumulate in fp32 PSUM.

Layouts on chip (per core):
  xT   [1024, 2048]  x[b].T, host-pretransposed
  QT,KT [512, 2048]  per-head-group q/k features x tokens (8 tiles [128,2048])
  V    [2048, 520]   tokens x (8 heads x (64 vals + ones col))
  S^T  [k, q] tiles  -> exp -> PV^T accumulation gives [65, q] per head
                       (row 64 = softmax denominator via the ones column)
  AoT  [512, 2048]   normalized attention output (features x tokens)
  y    [2048, 1024]  partial output projection (natural layout)
"""
import sys

sys.path.insert(0, "/opt/trn_rl_repo")

import numpy as np

D_MODEL = 1024
N_HEADS = 16
B = 4
T = 2048
HD = 64
N_CORES = 8
NH_LOC = N_HEADS // 2  # heads per core
FQ = NH_LOC * HD  # 512 local features

_prog_cache = {}


def build_program(tok=T):
    """Build the single-core SPMD Bass program. tok must be a multiple of 512."""
    import concourse.mybir as mybir
    import concourse.tile as tile
    from concourse import bacc

    f32 = mybir.dt.float32
    bf16 = mybir.dt.bfloat16
    f32r = mybir.dt.float32r
    P = 128
    QC = 512  # q-chunk width
    KC = D_MODEL // P  # 8 d-model chunks
    TT = tok // P  # token tiles
    NJ = tok // QC  # q-chunks
    NDC = FQ // P  # 4 din chunks

    nc = bacc.Bacc("TRN2", target_bir_lowering=False, debug=False, num_devices=N_CORES)

    xT = nc.dram_tensor("xT", [D_MODEL, tok], f32r, kind="ExternalInput")
    wq = nc.dram_tensor("wq", [D_MODEL, FQ], f32r, kind="ExternalInput")
    wk = nc.dram_tensor("wk", [D_MODEL, FQ], f32r, kind="ExternalInput")
    wv = nc.dram_tensor("wv", [D_MODEL, FQ], f32r, kind="ExternalInput")
    wo = nc.dram_tensor("wo", [FQ, D_MODEL], f32r, kind="ExternalInput")
    y = nc.dram_tensor("y", [tok, D_MODEL], f32, kind="ExternalOutput")

    with tile.TileContext(nc) as tc:
        with (
            tc.tile_pool(name="qkt", bufs=1) as qktp,
            tc.tile_pool(name="vp", bufs=1) as vp,
            tc.tile_pool(name="mask", bufs=1) as maskp,
            # shared PSUM pools for all phases: no pool-transition barrier,
            # so the PE never sees a multi-us gap (keeps HAM at 8/8)
            tc.tile_pool(name="big", bufs=3, space="PSUM") as bigp,
            tc.tile_pool(name="small", bufs=2, space="PSUM") as smallp,
        ):
            # persistent tiles
            QKT = [qktp.tile([P, tok], f32r, tag=f"qkt{i}", name=f"qkt{i}") for i in range(8)]
            V = [vp.tile([P, NH_LOC * (HD + 1)], f32r, tag=f"v{i}", name=f"v{i}") for i in range(TT)]

            # single causal mask triangle, duplicated for the two heads:
            # mask[p, q] = 0 where q >= p else -1e30, shape [128, 2*128]
            cmask = maskp.tile([P, 2 * P], bf16, tag="cmask", name="cmask")
            nc.gpsimd.memset(cmask[:], 0.0)
            for half in (0, 1):
                nc.gpsimd.affine_select(
                    out=cmask[:, half * P : (half + 1) * P],
                    in_=cmask[:, half * P : (half + 1) * P],
                    compare_op=mybir.AluOpType.is_ge,
                    fill=-1e30,
                    base=0,
                    pattern=[[1, P]],
                    channel_multiplier=-1,
                )

            # ---------------- Phase 1: projections ----------------
            with (
                tc.tile_pool(name="xt", bufs=1) as xtp,
                tc.tile_pool(name="wst", bufs=2) as wp,
                tc.tile_pool(name="wvp", bufs=1) as wvp,
            ):
                XT = [xtp.tile([P, tok], f32r, tag=f"xt{l}", name=f"xt{l}") for l in range(KC)]
                WV = [wvp.tile([P, FQ], f32r, tag=f"wv{l}", name=f"wv{l}") for l in range(KC)]
                WSL = {}
                # DMA issue order matters on the sync queue: the first two
                # weight slices come first so the PE can start immediately
                # after the first xT tiles land
                for ft in (0, 1):
                    WSL[ft] = wp.tile([P, KC * P], f32r, tag="w", name=f"wsl{ft}")
                    nc.sync.dma_start(
                        out=WSL[ft][:].rearrange("p (l f) -> p l f", f=P),
                        in_=wq[:, ft * P : (ft + 1) * P].rearrange(
                            "(l p) f -> p l f", p=P
                        ),
                    )
                for l in range(KC):
                    nc.sync.dma_start(out=XT[l][:], in_=xT[l * P : (l + 1) * P, :])
                for l in range(KC):
                    nc.sync.dma_start(out=WV[l][:], in_=wv[l * P : (l + 1) * P, :])

                # Q^T and K^T: out[feat, tok]; lhsT = w chunk, rhs = xT chunk
                for ft in range(8):
                    wsrc = wq if ft < 4 else wk
                    fo = (ft % 4) * P
                    if ft in WSL:
                        wsl = WSL[ft]
                    else:
                        wsl = wp.tile([P, KC * P], f32r, tag="w", name=f"wsl{ft}")
                        nc.sync.dma_start(
                            out=wsl[:].rearrange("p (l f) -> p l f", f=P),
                            in_=wsrc[:, fo : fo + P].rearrange("(l p) f -> p l f", p=P),
                        )
                    qs = list(range(NJ))
                    pbs = [
                        bigp.tile([P, 2 * QC], f32, tag="big", name="pqk")
                        for _ in range((NJ + 1) // 2)
                    ]
                    half = {
                        q: pbs[q // 2][:, (q % 2) * QC : (q % 2 + 1) * QC] for q in qs
                    }
                    for l in range(KC):
                        for q in qs:
                            nc.tensor.matmul(
                                half[q],
                                wsl[:, l * P : (l + 1) * P],
                                XT[l][:, q * QC : (q + 1) * QC],
                                start=(l == 0),
                                stop=(l == KC - 1),
                            )
                    for q in qs:
                        nc.scalar.copy(QKT[ft][:, q * QC : (q + 1) * QC], half[q])

                # V: out[tok, feat]; lhsT = xT chunk, rhs = wv chunk
                for tt in range(TT):
                    nc.gpsimd.memset(V[tt][:].bitcast(f32), 1.0)
                    psv = smallp.tile([P, FQ], f32, tag="small", name="psv")
                    for l in range(KC):
                        nc.tensor.matmul(
                            psv[:],
                            XT[l][:, tt * P : (tt + 1) * P],
                            WV[l][:],
                            start=(l == 0),
                            stop=(l == KC - 1),
                        )
                    # strided copy into the 65-col head groups (ones col stays)
                    vdst = V[tt][:].rearrange("p (u c) -> p u c", c=HD + 1)[
                        :, :, 0:HD
                    ]
                    vsrc = psv[:].rearrange("p (u c) -> p u c", c=HD)
                    nc.scalar.copy(vdst, vsrc)

            # ---------------- Phase 2: causal attention ----------------
            with (
                tc.tile_pool(name="aot", bufs=1) as aotp,
                tc.tile_pool(name="exp", bufs=4) as expp,
                tc.tile_pool(name="nrm", bufs=4) as nrmp,
                tc.tile_pool(name="wop", bufs=1) as wop,
                tc.tile_pool(name="yp", bufs=4) as yp,
            ):
                AOT = [aotp.tile([P, tok], f32r, tag=f"aot{d}", name=f"aot{d}") for d in range(NDC)]
                WO = [
                    wop.tile([P, D_MODEL], f32r, tag=f"wo{d}", name=f"wo{d}") for d in range(NDC)
                ]
                for d in range(NDC):
                    nc.sync.dma_start(out=WO[d][:], in_=wo[d * P : (d + 1) * P, :])

                for j in range(NJ):
                    for hp in range(NH_LOC // 2):
                        nkt = 4 * j + 4  # k-tiles for this q-chunk
                        uA, uB = 2 * hp, 2 * hp + 1
                        us = (uA, uB)
                        pv = {u: smallp.tile([HD + 1, QC], f32, tag="small", name=f"pv{u}") for u in us}
                        for i in range(nkt):
                            # one k-tile per step; both heads packed into one
                            # [128, 1024] PSUM tile (A: cols 0:512, B: 512:1024)
                            # -> 3 steps in flight with bufs=3, and the two
                            # 64-row ST matmuls alternate PE row groups.
                            # Diagonal k-tiles (s = i - 4j >= 0) only touch
                            # q >= 128*s, so all work shrinks to the window
                            # [128*s : 512] and the mask-add reduces to one
                            # shared 128-wide causal triangle at the window
                            # start.
                            s = i - 4 * j
                            w0 = 128 * s if s >= 0 else 0
                            wn = QC - w0
                            st = bigp.tile([P, 2 * QC], f32, tag="big", name="st")
                            for idx, u in enumerate(us):
                                rs = slice(64 * (u % 2), 64 * (u % 2) + 64)
                                nc.tensor.matmul(
                                    st[:, idx * QC + w0 : (idx + 1) * QC],
                                    QKT[4 + u // 2][rs, i * P : (i + 1) * P],
                                    QKT[u // 2][rs, j * QC + w0 : (j + 1) * QC],
                                    start=True,
                                    stop=True,
                                )
                            win3 = st[:].rearrange("p (h q) -> p h q", h=2)
                            if s >= 0:
                                nc.vector.tensor_tensor(
                                    out=win3[:, :, w0 : w0 + P],
                                    in0=win3[:, :, w0 : w0 + P],
                                    in1=cmask[:].rearrange(
                                        "p (h q) -> p h q", h=2
                                    ),
                                    op=mybir.AluOpType.add,
                                )
                            e = expp.tile([P, 2 * QC], f32r, tag="e", name="e")
                            nc.scalar.activation(
                                e[:].rearrange("p (h q) -> p h q", h=2)[
                                    :, :, w0:QC
                                ],
                                win3[:, :, w0:QC],
                                mybir.ActivationFunctionType.Exp,
                                scale=0.125,
                            )
                            for idx, u in enumerate(us):
                                nc.tensor.matmul(
                                    pv[u][:, w0:QC],
                                    V[i][:, u * (HD + 1) : (u + 1) * (HD + 1)],
                                    e[:, idx * QC + w0 : (idx + 1) * QC],
                                    start=(i == 0),
                                    stop=(i == nkt - 1),
                                )
                        for u in us:
                            # copy PSUM out fast to free the pv slot, then
                            # broadcast the denominator, reciprocal on 64
                            # lanes, and normalize
                            sa = nrmp.tile([HD, QC], f32, tag="sa", name="sa")
                            nc.vector.tensor_copy(sa[:], pv[u][0:HD, :])
                            sd = nrmp.tile([1, QC], f32, tag="sd", name="sd")
                            nc.vector.tensor_copy(sd[:], pv[u][HD : HD + 1, :])
                            bc = nrmp.tile([HD, QC], f32, tag="bc", name="bc")
                            nc.gpsimd.partition_broadcast(bc[:], sd[:])
                            nc.vector.reciprocal_approx_fast(bc[:], bc[:])
                            nc.vector.tensor_tensor(
                                out=AOT[u // 2][
                                    64 * (u % 2) : 64 * (u % 2) + 64,
                                    j * QC : (j + 1) * QC,
                                ],
                                in0=sa[:],
                                in1=bc[:],
                                op=mybir.AluOpType.mult,
                            )

                # ---------------- Phase 3: output projection ----------------
                for tt in range(TT):
                    pb = bigp.tile([P, 2 * QC], f32, tag="big", name="py")
                    for d in range(NDC):
                        for h in (0, 1):
                            nc.tensor.matmul(
                                pb[:, h * QC : (h + 1) * QC],
                                AOT[d][:, tt * P : (tt + 1) * P],
                                WO[d][:, h * QC : (h + 1) * QC],
                                start=(d == 0),
                                stop=(d == NDC - 1),
                            )
                    for h in (0, 1):
                        ysb = yp.tile([P, QC], f32, tag="y")
                        nc.scalar.copy(ysb[:], pb[:, h * QC : (h + 1) * QC])
                        nc.sync.dma_start(
                            out=y[tt * P : (tt + 1) * P, h * QC : (h + 1) * QC],
                            in_=ysb[:],
                        )
    nc.compile()
    return nc


def get_program(tok=T):
    if tok not in _prog_cache:
        _prog_cache[tok] = build_program(tok)
    return _prog_cache[tok]


def make_in_maps(x, w_qkv, w_out):
    """Shard full inputs into 8 per-core input maps."""
    x = np.asarray(x, dtype=np.float32)
    w_qkv = np.asarray(w_qkv, dtype=np.float32)
    w_out = np.asarray(w_out, dtype=np.float32)
    D = D_MODEL
    xTs = [np.ascontiguousarray(x[b].T) for b in range(x.shape[0])]
    in_maps = []
    for c in range(N_CORES):
        b, hg = c // 2, c % 2
        in_maps.append(
            {
                "xT": xTs[b],
                "wq": np.ascontiguousarray(w_qkv[:, hg * FQ : (hg + 1) * FQ]),
                "wk": np.ascontiguousarray(
                    w_qkv[:, D + hg * FQ : D + (hg + 1) * FQ]
                ),
                "wv": np.ascontiguousarray(
                    w_qkv[:, 2 * D + hg * FQ : 2 * D + (hg + 1) * FQ]
                ),
                "wo": np.ascontiguousarray(w_out[hg * FQ : (hg + 1) * FQ, :]),
            }
        )
    return in_maps


_runner_cache = {}


def _make_runner(nc, n_cores=N_CORES):
    """Cached multi-core executor (same semantics as bass2jax.run_bass_via_pjrt
    for a program with no partition-id and no debug tensors, but the jitted
    callable is reusable so repeat kernel() calls don't recompile)."""
    import jax
    from jax.sharding import Mesh, PartitionSpec
    from jax.experimental.shard_map import shard_map
    import concourse.mybir as mybir
    from concourse.bass2jax import _bass_exec_p, install_neuronx_cc_hook

    install_neuronx_cc_hook()

    in_names, out_names, out_avals = [], [], []
    for alloc in nc.m.functions[0].allocations:
        if not isinstance(alloc, mybir.MemoryLocationSet):
            continue
        name = alloc.memorylocations[0].name
        if alloc.kind == "ExternalInput":
            in_names.append(name)
        elif alloc.kind == "ExternalOutput":
            out_names.append(name)
            out_avals.append(
                jax.core.ShapedArray(
                    tuple(alloc.tensor_shape), mybir.dt.np(alloc.dtype)
                )
            )
    n_params = len(in_names)
    n_outs = len(out_avals)
    all_in_names = in_names + out_names

    def _body(*args):
        outs = _bass_exec_p.bind(
            *args,
            out_avals=tuple(out_avals),
            in_names=tuple(all_in_names),
            out_names=tuple(out_names),
            lowering_input_output_aliases=(),
            sim_require_finite=True,
            sim_require_nnan=True,
            nc=nc,
        )
        return tuple(outs)

    devices = jax.devices()[:n_cores]
    mesh = Mesh(np.asarray(devices), ("core",))
    donate = tuple(range(n_params, n_params + n_outs))
    sharded = jax.jit(
        shard_map(
            _body,
            mesh=mesh,
            in_specs=(PartitionSpec("core"),) * (n_params + n_outs),
            out_specs=(PartitionSpec("core"),) * n_outs,
            check_rep=False,
        ),
        donate_argnums=donate,
        keep_unused=True,
    )

    def run(in_maps):
        per_core = [[np.asarray(m[nm]) for nm in in_names] for m in in_maps]
        concat_in = [
            np.concatenate([per_core[c][i] for c in range(n_cores)], axis=0)
            for i in range(n_params)
        ]
        concat_zeros = [
            np.zeros((n_cores * a.shape[0], *a.shape[1:]), a.dtype)
            for a in out_avals
        ]
        out_arrs = sharded(*concat_in, *concat_zeros)
        return [
            {
                nm: np.asarray(out_arrs[i]).reshape(n_cores, *out_avals[i].shape)[c]
                for i, nm in enumerate(out_names)
            }
            for c in range(n_cores)
        ]

    return run


def get_runner(tok=T):
    if tok not in _runner_cache:
        _runner_cache[tok] = _make_runner(get_program(tok))
    return _runner_cache[tok]


def kernel(x, w_qkv, w_out, b_out):
    in_maps = make_in_maps(x, w_qkv, w_out)
    try:
        run = get_runner(T)
        results = run(in_maps)
    except Exception:
        # fallback: the stock SPMD runner (recompiles per call but is the
        # battle-tested path)
        from concourse.bass_utils import run_bass_kernel_spmd

        results = run_bass_kernel_spmd(
            get_program(T), in_maps, list(range(N_CORES))
        ).results
    b_out = np.asarray(b_out, dtype=np.float32)
    out = np.empty((B, T, D_MODEL), dtype=np.float32)
    for b in range(B):
        out[b] = results[2 * b]["y"] + results[2 * b + 1]["y"] + b_out
    return out



# revision 15
# speedup vs baseline: 1.0295x; 1.0295x over previous
"""Causal self-attention TRN2 Bass kernel (bf16, software-pipelined).

Sharding: 8 cores = 4 batches x 2 head-groups. Core c handles batch c//2 and
heads (c%2)*8 .. (c%2)*8+8 (of 16). Each core computes its heads' attention
and a partial output projection; the host sums the two partials per batch and
adds b_out.

v2 design (vs f32r baseline):
  - all matmul operands bf16 (FWL weight loads overlap the stream; f32r
    self-loading matmuls serialized a ~180ns weight load per matmul)
  - single pool scope, one long instruction stream: QK/V/out projections are
    emitted as "filler" chains interleaved into the ACT-bound attention
    cadence, so the PE never idles and HAM stays at 8/8
  - ACT does exp only; PSUM evictions ride on DVE (casts) and Pool (y copies)
  - softmax denominator via the ones-column of V (row 64 of the PV PSUM);
    normalization = reciprocal + partition_broadcast + fused multiply-evict

Layouts on chip (per core):
  XT    8 x [128, 2048] bf16   x[b].T d-major tiles
  WQ/WK/WV 8 x [128, 512] bf16, WO 4 x [128, 1024] bf16 (contiguous rows)
  QKT   8 x [128, 2048] bf16   Q^T (0..3) / K^T (4..7) features x tokens
  V     16 x [128, 520] bf16   tokens x (8 heads x (64 vals + ones col))
  e     [128, 1024] bf16       exp(S^T) per k-tile, both heads
  AOT   4 x [128, 2048] bf16   normalized attention out (features x tokens)
  y     [2048, 1024] f32       partial output projection
"""
import sys

sys.path.insert(0, "/opt/trn_rl_repo")

import numpy as np
import ml_dtypes

D_MODEL = 1024
N_HEADS = 16
B = 4
T = 2048
HD = 64
N_CORES = 8
NH_LOC = N_HEADS // 2  # heads per core
FQ = NH_LOC * HD  # 512 local features

_prog_cache = {}


def build_program(tok=T, debug_dumps=False):
    """Build the single-core SPMD Bass program. tok must be a multiple of 512."""
    import concourse.mybir as mybir
    import concourse.tile as tile
    from concourse import bacc

    f32 = mybir.dt.float32
    bf16 = mybir.dt.bfloat16
    P = 128
    QC = 512  # q-chunk width
    KC = D_MODEL // P  # 8 d-model chunks
    TT = tok // P  # token tiles
    NJ = tok // QC  # q-chunks
    NDC = FQ // P  # 4 feature chunks

    nc = bacc.Bacc("TRN2", target_bir_lowering=False, debug=False, num_devices=N_CORES)

    xT = nc.dram_tensor("xT", [D_MODEL, tok], bf16, kind="ExternalInput")
    wq = nc.dram_tensor("wq", [D_MODEL, FQ], bf16, kind="ExternalInput")
    wk = nc.dram_tensor("wk", [D_MODEL, FQ], bf16, kind="ExternalInput")
    wv = nc.dram_tensor("wv", [D_MODEL, FQ], bf16, kind="ExternalInput")
    wo = nc.dram_tensor("wo", [FQ, D_MODEL], bf16, kind="ExternalInput")
    y = nc.dram_tensor("y", [tok, D_MODEL], f32, kind="ExternalOutput")
    if debug_dumps:
        dbg_qkt = nc.dram_tensor("dbg_qkt", [8 * 128, tok], bf16, kind="ExternalOutput")
        dbg_v = nc.dram_tensor("dbg_v", [2 * 128, NH_LOC * (HD + 1)], bf16, kind="ExternalOutput")
        dbg_aot = nc.dram_tensor("dbg_aot", [4 * 128, tok], bf16, kind="ExternalOutput")
        dbg_st = nc.dram_tensor("dbg_st", [128, 1024], f32, kind="ExternalOutput")
        dbg_e = nc.dram_tensor("dbg_e", [128, 1024], bf16, kind="ExternalOutput")
        dbg_pv = nc.dram_tensor("dbg_pv", [HD + 1, QC], f32, kind="ExternalOutput")

    with tile.TileContext(nc) as tc:
        with (
            tc.tile_pool(name="wqp", bufs=1) as wqp,
            tc.tile_pool(name="wkp", bufs=1) as wkp,
            tc.tile_pool(name="wvp", bufs=1) as wvp,
            tc.tile_pool(name="wop", bufs=1) as wop,
            tc.tile_pool(name="xtp", bufs=1) as xtp,
            tc.tile_pool(name="qktp", bufs=1) as qktp,
            tc.tile_pool(name="vp", bufs=1) as vp,
            tc.tile_pool(name="aotp", bufs=1) as aotp,
            tc.tile_pool(name="ep", bufs=3) as ep,
            tc.tile_pool(name="ystp", bufs=3) as ystp,
            tc.tile_pool(name="mvp", bufs=1) as mvp,
            tc.tile_pool(name="nrm", bufs=4) as nrmp,
            tc.tile_pool(name="big", bufs=2, space="PSUM") as bigp,   # 4 banks
            tc.tile_pool(name="pvp", bufs=2, space="PSUM") as pvp,    # 2 banks
            tc.tile_pool(name="prj", bufs=2, space="PSUM") as prjp,   # 2 banks
        ):
            WQ = [wqp.tile([P, FQ], bf16, tag=f"wq{l}", name=f"wq{l}") for l in range(KC)]
            WK = [wkp.tile([P, FQ], bf16, tag=f"wk{l}", name=f"wk{l}") for l in range(KC)]
            WV = [wvp.tile([P, FQ], bf16, tag=f"wv{l}", name=f"wv{l}") for l in range(KC)]
            WO = [wop.tile([P, D_MODEL], bf16, tag=f"wo{d}", name=f"wo{d}") for d in range(NDC)]
            XT = [xtp.tile([P, tok], bf16, tag=f"xt{l}", name=f"xt{l}") for l in range(KC)]
            QKT = [qktp.tile([P, tok], bf16, tag=f"qkt{i}", name=f"qkt{i}") for i in range(8)]
            V = [vp.tile([P, NH_LOC * (HD + 1)], bf16, tag=f"v{i}", name=f"v{i}") for i in range(TT)]
            AOT = [aotp.tile([P, tok], bf16, tag=f"aot{d}", name=f"aot{d}") for d in range(NDC)]

            # warm the exp table while input DMAs stream
            warm = mvp.tile([1, 8], f32, tag="warm", name="warm")
            nc.gpsimd.memset(warm[:], 0.0)
            nc.scalar.activation(warm[:], warm[:], mybir.ActivationFunctionType.Exp)

            # causal mask triangle, duplicated for the two heads:
            # cmask[p, q] = 0 where q >= p else -1e30, shape [128, 2*128]
            cmask = mvp.tile([P, 2 * P], bf16, tag="cmask", name="cmask")
            nc.gpsimd.memset(cmask[:], 0.0)
            for half in (0, 1):
                nc.gpsimd.affine_select(
                    out=cmask[:, half * P : (half + 1) * P],
                    in_=cmask[:, half * P : (half + 1) * P],
                    compare_op=mybir.AluOpType.is_ge,
                    fill=-1e30,
                    base=0,
                    pattern=[[1, P]],
                    channel_multiplier=-1,
                )
            # ones columns of V: memset the whole tile (value cols are
            # overwritten by the projection eviction; col 64 of each 65-wide
            # head group keeps the 1.0)
            for tt in range(TT):
                nc.gpsimd.memset(V[tt][:], 1.0)

            # ---------------- input DMAs ----------------
            # order: wq, xt (c0 chunks first), wk, xt rest, wv, wo — so the
            # first QT chains can start ~2us in
            for l in range(KC):
                nc.sync.dma_start(out=WQ[l][:], in_=wq[l * P : (l + 1) * P, :])
            for l in range(KC):
                nc.sync.dma_start(out=XT[l][:, 0:QC], in_=xT[l * P : (l + 1) * P, 0:QC])
            for l in range(KC):
                nc.sync.dma_start(out=WK[l][:], in_=wk[l * P : (l + 1) * P, :])
            for c in range(1, NJ):
                for l in range(KC):
                    nc.sync.dma_start(
                        out=XT[l][:, c * QC : (c + 1) * QC],
                        in_=xT[l * P : (l + 1) * P, c * QC : (c + 1) * QC],
                    )
            for l in range(KC):
                nc.sync.dma_start(out=WV[l][:], in_=wv[l * P : (l + 1) * P, :])
            for d in range(NDC):
                nc.sync.dma_start(out=WO[d][:], in_=wo[d * P : (d + 1) * P, :])

            # ---------------- filler chain emitters ----------------
            def emit_qk_chain(ft, c):
                """QKT[ft][:, c*QC:(c+1)*QC] = (w-slice)^T @ XT, 8 MMs + copy."""
                wsrc = WQ if ft < 4 else WK
                fo = (ft % 4) * P
                p = prjp.tile([P, QC], f32, tag="prj", name=f"pqk{ft}_{c}")
                for l in range(KC):
                    nc.tensor.matmul(
                        p[:],
                        wsrc[l][:, fo : fo + P],
                        XT[l][:, c * QC : (c + 1) * QC],
                        start=(l == 0),
                        stop=(l == KC - 1),
                    )
                nc.vector.tensor_copy(QKT[ft][:, c * QC : (c + 1) * QC], p[:])

            def emit_v_chain(tt):
                """V[tt] value cols = XT-slice^T @ WV, 8 MMs + strided copy."""
                p = prjp.tile([P, FQ], f32, tag="prj", name=f"pv{tt}")
                for l in range(KC):
                    nc.tensor.matmul(
                        p[:],
                        XT[l][:, tt * P : (tt + 1) * P],
                        WV[l][:],
                        start=(l == 0),
                        stop=(l == KC - 1),
                    )
                vdst = V[tt][:].rearrange("p (u c) -> p u c", c=HD + 1)[:, :, 0:HD]
                vsrc = p[:].rearrange("p (u c) -> p u c", c=HD)
                nc.vector.tensor_copy(vdst, vsrc)

            def emit_out_chain(tt, h):
                """y[tt-tile, h-half] = AOT-slice^T @ WO, 4 MMs + copy + DMA."""
                p = prjp.tile([P, QC], f32, tag="prj", name=f"py{tt}_{h}")
                for d in range(NDC):
                    nc.tensor.matmul(
                        p[:],
                        AOT[d][:, tt * P : (tt + 1) * P],
                        WO[d][:, h * QC : (h + 1) * QC],
                        start=(d == 0),
                        stop=(d == NDC - 1),
                    )
                ysb = ystp.tile([P, QC], f32, tag="y")
                nc.vector.tensor_copy(ysb[:], p[:])
                nc.sync.dma_start(
                    out=y[tt * P : (tt + 1) * P, h * QC : (h + 1) * QC],
                    in_=ysb[:],
                )

            # per-hp filler queues: V chains front-loaded during hp0 (V[tt]
            # is consumed by PV at k-tile tt), remaining QK projections paced
            # across each hp's attention so QKT[hp+1] is ready in time
            filler_q = {hp: [] for hp in range(4)}
            for k, tt in enumerate(range(4, TT)):
                filler_q[0].append(lambda tt=tt: emit_v_chain(tt))
            for hp in range(1, 4):
                for ft in (hp, 4 + hp):
                    for c in range(NJ):
                        filler_q[hp - 1].append(
                            lambda ft=ft, c=c: emit_qk_chain(ft, c)
                        )

            # ---------------- pre-attention work ----------------
            for ft in (0, 4):
                for c in range(NJ):
                    emit_qk_chain(ft, c)
            for tt in range(4):
                emit_v_chain(tt)

            # ---------------- attention + interleaved fillers ----------------
            out_ready = []  # out-proj chains unlocked so far
            for hp in range(4):
                fq = filler_q[hp]
                fq.reverse()  # pop() in front order
                nsteps_hp = sum(4 * j + 4 for j in range(NJ))  # 40
                credit = 0.0
                rate = len(fq) / (nsteps_hp * 0.85)
                for j in range(NJ):
                    nkt = 4 * j + 4
                    pv = {
                        u: pvp.tile([HD + 1, QC], f32, tag="pv", name=f"pv{u}")
                        for u in (0, 1)
                    }
                    for i in range(nkt):
                        s = i - 4 * j
                        w0 = 128 * s if s >= 0 else 0
                        st = bigp.tile([P, 2 * QC], f32, tag="big", name="st")
                        for u in (0, 1):
                            rs = slice(64 * u, 64 * u + 64)
                            nc.tensor.matmul(
                                st[:, u * QC + w0 : (u + 1) * QC],
                                QKT[4 + hp][rs, i * P : (i + 1) * P],
                                QKT[hp][rs, j * QC + w0 : (j + 1) * QC],
                                start=True,
                                stop=True,
                            )
                        win3 = st[:].rearrange("p (h q) -> p h q", h=2)
                        if s >= 0:
                            nc.vector.tensor_tensor(
                                out=win3[:, :, w0 : w0 + P],
                                in0=win3[:, :, w0 : w0 + P],
                                in1=cmask[:].rearrange("p (h q) -> p h q", h=2),
                                op=mybir.AluOpType.add,
                            )
                        e = ep.tile([P, 2 * QC], bf16, tag="e", name="e")
                        nc.scalar.activation(
                            e[:].rearrange("p (h q) -> p h q", h=2)[:, :, w0:QC],
                            win3[:, :, w0:QC],
                            mybir.ActivationFunctionType.Exp,
                            scale=0.125,
                        )
                        if debug_dumps and hp == 0 and j == 0 and i == 0:
                            stsb = ystp.tile([P, 2 * QC], f32, tag="dbgst", name="dbgst")
                            nc.vector.tensor_copy(stsb[:], st[:])
                            nc.sync.dma_start(out=dbg_st[:, :], in_=stsb[:])
                            nc.sync.dma_start(out=dbg_e[:, :], in_=e[:])
                        for u in (0, 1):
                            hloc = 2 * hp + u
                            nc.tensor.matmul(
                                pv[u][:, w0:QC],
                                V[i][:, hloc * (HD + 1) : (hloc + 1) * (HD + 1)],
                                e[:, u * QC + w0 : (u + 1) * QC],
                                start=(i == 0),
                                stop=(i == nkt - 1),
                            )
                        credit += rate
                        while fq and credit >= 1.0:
                            fq.pop()()
                            credit -= 1.0
                        # out-proj chains become fillers once unlocked
                        if out_ready:
                            out_ready.pop(0)()
                    if debug_dumps and hp == 0 and j == 0:
                        pvsb = ystp.tile([HD + 1, QC], f32, tag="dbgpv", name="dbgpv")
                        nc.vector.tensor_copy(pvsb[:], pv[0][:])
                        nc.sync.dma_start(out=dbg_pv[:, :], in_=pvsb[:])
                    for u in (0, 1):
                        # normalization (baseline-proven sequence): copy the
                        # accumulator + denominator row out of PSUM, broadcast
                        # the denominator, reciprocal, multiply into AOT
                        sa = nrmp.tile([HD, QC], f32, tag="sa", name="sa")
                        nc.vector.tensor_copy(sa[:], pv[u][0:HD, :])
                        sd = nrmp.tile([1, QC], f32, tag="sd", name="sd")
                        nc.vector.tensor_copy(sd[:], pv[u][HD : HD + 1, :])
                        bc = nrmp.tile([HD, QC], f32, tag="bc", name="bc")
                        nc.gpsimd.partition_broadcast(bc[:], sd[:])
                        nc.vector.reciprocal_approx_fast(bc[:], bc[:])
                        nc.vector.tensor_tensor(
                            out=AOT[hp][
                                64 * u : 64 * u + 64, j * QC : (j + 1) * QC
                            ],
                            in0=sa[:],
                            in1=bc[:],
                            op=mybir.AluOpType.mult,
                        )
                    # after the last head-pair finishes chunk j, its tokens'
                    # output projection is unlocked
                    if hp == 3:
                        for tt in range(4 * j, 4 * j + 4):
                            for h in (0, 1):
                                out_ready.append(
                                    lambda tt=tt, h=h: emit_out_chain(tt, h)
                                )
                # drain this hp's fillers before moving to the next head-pair
                while fq:
                    fq.pop()()
            # drain remaining out-proj chains
            while out_ready:
                out_ready.pop(0)()
            if debug_dumps:
                for ft in range(8):
                    nc.sync.dma_start(
                        out=dbg_qkt[ft * 128 : (ft + 1) * 128, :], in_=QKT[ft][:]
                    )
                for tt in range(2):
                    nc.sync.dma_start(
                        out=dbg_v[tt * 128 : (tt + 1) * 128, :], in_=V[tt][:]
                    )
                for d in range(4):
                    nc.sync.dma_start(
                        out=dbg_aot[d * 128 : (d + 1) * 128, :], in_=AOT[d][:]
                    )
    nc.compile()
    return nc


def get_program(tok=T):
    if tok not in _prog_cache:
        _prog_cache[tok] = build_program(tok)
    return _prog_cache[tok]


def make_in_maps(x, w_qkv, w_out):
    """Shard full inputs into 8 per-core input maps (bf16)."""
    bf = ml_dtypes.bfloat16
    x = np.asarray(x, dtype=np.float32)
    w_qkv = np.asarray(w_qkv, dtype=np.float32).astype(bf)
    w_out = np.asarray(w_out, dtype=np.float32).astype(bf)
    D = D_MODEL
    xTs = [np.ascontiguousarray(x[b].T).astype(bf) for b in range(x.shape[0])]
    in_maps = []
    for c in range(N_CORES):
        b, hg = c // 2, c % 2
        in_maps.append(
            {
                "xT": xTs[b],
                "wq": np.ascontiguousarray(w_qkv[:, hg * FQ : (hg + 1) * FQ]),
                "wk": np.ascontiguousarray(
                    w_qkv[:, D + hg * FQ : D + (hg + 1) * FQ]
                ),
                "wv": np.ascontiguousarray(
                    w_qkv[:, 2 * D + hg * FQ : 2 * D + (hg + 1) * FQ]
                ),
                "wo": np.ascontiguousarray(w_out[hg * FQ : (hg + 1) * FQ, :]),
            }
        )
    return in_maps


_runner_cache = {}


def _make_runner(nc, n_cores=N_CORES):
    """Cached multi-core executor (same semantics as bass2jax.run_bass_via_pjrt
    for a program with no partition-id and no debug tensors, but the jitted
    callable is reusable so repeat kernel() calls don't recompile)."""
    import jax
    from jax.sharding import Mesh, PartitionSpec
    from jax.experimental.shard_map import shard_map
    import concourse.mybir as mybir
    from concourse.bass2jax import _bass_exec_p, install_neuronx_cc_hook

    install_neuronx_cc_hook()

    in_names, out_names, out_avals = [], [], []
    for alloc in nc.m.functions[0].allocations:
        if not isinstance(alloc, mybir.MemoryLocationSet):
            continue
        name = alloc.memorylocations[0].name
        if alloc.kind == "ExternalInput":
            in_names.append(name)
        elif alloc.kind == "ExternalOutput":
            out_names.append(name)
            out_avals.append(
                jax.core.ShapedArray(
                    tuple(alloc.tensor_shape), mybir.dt.np(alloc.dtype)
                )
            )
    n_params = len(in_names)
    n_outs = len(out_avals)
    all_in_names = in_names + out_names

    def _body(*args):
        outs = _bass_exec_p.bind(
            *args,
            out_avals=tuple(out_avals),
            in_names=tuple(all_in_names),
            out_names=tuple(out_names),
            lowering_input_output_aliases=(),
            sim_require_finite=True,
            sim_require_nnan=True,
            nc=nc,
        )
        return tuple(outs)

    devices = jax.devices()[:n_cores]
    mesh = Mesh(np.asarray(devices), ("core",))
    donate = tuple(range(n_params, n_params + n_outs))
    sharded = jax.jit(
        shard_map(
            _body,
            mesh=mesh,
            in_specs=(PartitionSpec("core"),) * (n_params + n_outs),
            out_specs=(PartitionSpec("core"),) * n_outs,
            check_rep=False,
        ),
        donate_argnums=donate,
        keep_unused=True,
    )

    def run(in_maps):
        per_core = [[np.asarray(m[nm]) for nm in in_names] for m in in_maps]
        concat_in = [
            np.concatenate([per_core[c][i] for c in range(n_cores)], axis=0)
            for i in range(n_params)
        ]
        concat_zeros = [
            np.zeros((n_cores * a.shape[0], *a.shape[1:]), a.dtype)
            for a in out_avals
        ]
        out_arrs = sharded(*concat_in, *concat_zeros)
        return [
            {
                nm: np.asarray(out_arrs[i]).reshape(n_cores, *out_avals[i].shape)[c]
                for i, nm in enumerate(out_names)
            }
            for c in range(n_cores)
        ]

    return run


def get_runner(tok=T):
    if tok not in _runner_cache:
        _runner_cache[tok] = _make_runner(get_program(tok))
    return _runner_cache[tok]


def kernel(x, w_qkv, w_out, b_out):
    in_maps = make_in_maps(x, w_qkv, w_out)
    try:
        run = get_runner(T)
        results = run(in_maps)
    except Exception:
        # fallback: the stock SPMD runner (recompiles per call but is the
        # battle-tested path)
        from concourse.bass_utils import run_bass_kernel_spmd

        results = run_bass_kernel_spmd(
            get_program(T), in_maps, list(range(N_CORES))
        ).results
    b_out = np.asarray(b_out, dtype=np.float32)
    out = np.empty((B, T, D_MODEL), dtype=np.float32)
    for b in range(B):
        out[b] = results[2 * b]["y"] + results[2 * b + 1]["y"] + b_out
    return out


# revision 16
# speedup vs baseline: 1.0308x; 1.0013x over previous
"""Causal self-attention TRN2 Bass kernel (bf16, software-pipelined).

Sharding: 8 cores = 4 batches x 2 head-groups. Core c handles batch c//2 and
heads (c%2)*8 .. (c%2)*8+8 (of 16). Each core computes its heads' attention
and a partial output projection; the host sums the two partials per batch and
adds b_out.

Design notes:
  - all matmul operands bf16 (FWL weight loads overlap the stream; f32r
    self-loading matmuls serialize a ~180ns weight load per matmul)
  - all DRAM inputs are host-packed to [128, N] partition-major layouts so
    every DMA moves 8-32KB contiguous rows (descriptor-rate limited otherwise)
  - single pool scope, one long instruction stream: QK/V/out projections are
    emitted as deadline-scheduled "filler" half-chains interleaved into the
    ACT-bound attention cadence, so the PE never idles and HAM stays at 8/8
  - softmax denominator via the ones-column of V (row 64 of the PV PSUM)

Layouts on chip (per core):
  XT    [128, 8x2048] bf16   x[b].T, d-major l-tiles (views XT[l])
  WQ/WK/WV [128, 8x512] bf16, WO [128, 4x1024] bf16
  QKT   8 x [128, 2048] bf16  Q^T (0..3) / K^T (4..7) features x tokens
  V     16 x [128, 520] bf16  tokens x (8 heads x (64 vals + ones col))
  e     [128, 1024] bf16      exp(S^T) per k-tile, both heads
  AOT   4 x [128, 2048] bf16  normalized attention out (features x tokens)
  y     [2048, 1024] f32      partial output projection
"""
import sys

sys.path.insert(0, "/opt/trn_rl_repo")

import numpy as np
import ml_dtypes

D_MODEL = 1024
N_HEADS = 16
B = 4
T = 2048
HD = 64
N_CORES = 8
NH_LOC = N_HEADS // 2  # heads per core
FQ = NH_LOC * HD  # 512 local features

_prog_cache = {}


def build_program(tok=T, debug_dumps=False):
    """Build the single-core SPMD Bass program. tok must be a multiple of 512."""
    import concourse.mybir as mybir
    import concourse.tile as tile
    from concourse import bacc

    f32 = mybir.dt.float32
    bf16 = mybir.dt.bfloat16
    P = 128
    QC = 512  # q-chunk width
    KC = D_MODEL // P  # 8 d-model chunks
    TT = tok // P  # token tiles
    NJ = tok // QC  # q-chunks
    NDC = FQ // P  # 4 feature chunks

    nc = bacc.Bacc("TRN2", target_bir_lowering=False, debug=False, num_devices=N_CORES)

    xT = nc.dram_tensor("xT", [P, KC * tok], bf16, kind="ExternalInput")
    wq = nc.dram_tensor("wq", [P, KC * FQ], bf16, kind="ExternalInput")
    wk = nc.dram_tensor("wk", [P, KC * FQ], bf16, kind="ExternalInput")
    wv = nc.dram_tensor("wv", [P, KC * FQ], bf16, kind="ExternalInput")
    wo = nc.dram_tensor("wo", [P, NDC * D_MODEL], bf16, kind="ExternalInput")
    y = nc.dram_tensor("y", [tok, D_MODEL], f32, kind="ExternalOutput")
    if debug_dumps:
        dbg_qkt = nc.dram_tensor("dbg_qkt", [8 * 128, tok], bf16, kind="ExternalOutput")
        dbg_v = nc.dram_tensor("dbg_v", [2 * 128, NH_LOC * (HD + 1)], bf16, kind="ExternalOutput")
        dbg_aot = nc.dram_tensor("dbg_aot", [4 * 128, tok], bf16, kind="ExternalOutput")

    with tile.TileContext(nc) as tc:
        with (
            tc.tile_pool(name="wqp", bufs=1) as wqp,
            tc.tile_pool(name="wkp", bufs=1) as wkp,
            tc.tile_pool(name="wvp", bufs=1) as wvp,
            tc.tile_pool(name="wop", bufs=1) as wop,
            tc.tile_pool(name="xtp", bufs=1) as xtp,
            tc.tile_pool(name="qktp", bufs=1) as qktp,
            tc.tile_pool(name="vp", bufs=1) as vp,
            tc.tile_pool(name="aotp", bufs=1) as aotp,
            tc.tile_pool(name="ep", bufs=3) as ep,
            tc.tile_pool(name="ystp", bufs=3) as ystp,
            tc.tile_pool(name="mvp", bufs=1) as mvp,
            tc.tile_pool(name="nrm", bufs=4) as nrmp,
            tc.tile_pool(name="big", bufs=2, space="PSUM") as bigp,   # 4 banks
            tc.tile_pool(name="pvp", bufs=2, space="PSUM") as pvp,    # 2 banks
            tc.tile_pool(name="prj", bufs=2, space="PSUM") as prjp,   # 2 banks
        ):
            WQa = wqp.tile([P, KC * FQ], bf16, tag="wq", name="wq")
            WKa = wkp.tile([P, KC * FQ], bf16, tag="wk", name="wk")
            WVa = wvp.tile([P, KC * FQ], bf16, tag="wv", name="wv")
            WOa = wop.tile([P, NDC * D_MODEL], bf16, tag="wo", name="wo")
            XTa = xtp.tile([P, KC * tok], bf16, tag="xt", name="xt")
            WQ = [WQa[:, l * FQ : (l + 1) * FQ] for l in range(KC)]
            WK = [WKa[:, l * FQ : (l + 1) * FQ] for l in range(KC)]
            WV = [WVa[:, l * FQ : (l + 1) * FQ] for l in range(KC)]
            WO = [WOa[:, d * D_MODEL : (d + 1) * D_MODEL] for d in range(NDC)]
            XT = [XTa[:, l * tok : (l + 1) * tok] for l in range(KC)]
            QKT = [qktp.tile([P, tok], bf16, tag=f"qkt{i}", name=f"qkt{i}") for i in range(8)]
            V = [vp.tile([P, NH_LOC * (HD + 1)], bf16, tag=f"v{i}", name=f"v{i}") for i in range(TT)]
            AOT = [aotp.tile([P, tok], bf16, tag=f"aot{d}", name=f"aot{d}") for d in range(NDC)]

            # ---------------- input DMAs ----------------
            # weights on the sync queue, x tiles on the gpsimd queue: the two
            # queues issue in parallel.  Splits sized so the first QK chains
            # can start ~2us in (chains walk l ascending).
            nc.sync.dma_start(out=WQa[:, : 4 * FQ], in_=wq[:, : 4 * FQ])
            nc.gpsimd.dma_start(out=XTa[:, : 2 * tok], in_=xT[:, : 2 * tok])
            nc.sync.dma_start(out=WQa[:, 4 * FQ :], in_=wq[:, 4 * FQ :])
            nc.gpsimd.dma_start(
                out=XTa[:, 2 * tok : 4 * tok], in_=xT[:, 2 * tok : 4 * tok]
            )
            nc.sync.dma_start(out=WKa[:], in_=wk[:])
            nc.gpsimd.dma_start(
                out=XTa[:, 4 * tok : 6 * tok], in_=xT[:, 4 * tok : 6 * tok]
            )
            nc.sync.dma_start(out=WVa[:], in_=wv[:])
            nc.gpsimd.dma_start(out=XTa[:, 6 * tok :], in_=xT[:, 6 * tok :])
            nc.sync.dma_start(out=WOa[:], in_=wo[:])

            # warm the exp table while input DMAs stream
            warm = mvp.tile([1, 8], f32, tag="warm", name="warm")
            nc.gpsimd.memset(warm[:], 0.0)
            nc.scalar.activation(warm[:], warm[:], mybir.ActivationFunctionType.Exp)

            # causal mask triangle, duplicated for the two heads:
            # cmask[p, q] = 0 where q >= p else -1e30, shape [128, 2*128]
            cmask = mvp.tile([P, 2 * P], bf16, tag="cmask", name="cmask")
            nc.gpsimd.memset(cmask[:], 0.0)
            for half in (0, 1):
                nc.gpsimd.affine_select(
                    out=cmask[:, half * P : (half + 1) * P],
                    in_=cmask[:, half * P : (half + 1) * P],
                    compare_op=mybir.AluOpType.is_ge,
                    fill=-1e30,
                    base=0,
                    pattern=[[1, P]],
                    channel_multiplier=-1,
                )
            # ones columns of V: memset whole tiles (value cols overwritten by
            # the projection eviction; col 64 of each head group keeps 1.0)
            for tt in range(TT):
                nc.gpsimd.memset(V[tt][:], 1.0)

            # ---------------- filler chains (emitted in halves) ----------------
            open_chains = {}

            def qk_half(ft, c, part):
                """QKT[ft][:, c-chunk] = (w-slice)^T @ XT over l; 2 halves."""
                wsrc = WQ if ft < 4 else WK
                fo = (ft % 4) * P
                key = ("qk", ft, c)
                if part == 0:
                    open_chains[key] = prjp.tile([P, QC], f32, tag="prj", name=f"pqk{ft}_{c}")
                p = open_chains[key]
                for l in range(4 * part, 4 * part + 4):
                    nc.tensor.matmul(
                        p[:],
                        wsrc[l][:, fo : fo + P],
                        XT[l][:, c * QC : (c + 1) * QC],
                        start=(l == 0),
                        stop=(l == KC - 1),
                    )
                if part == 1:
                    del open_chains[key]
                    nc.vector.tensor_copy(QKT[ft][:, c * QC : (c + 1) * QC], p[:])

            def v_half(tt, part):
                """V[tt] value cols = XT-slice^T @ WV; 2 halves."""
                key = ("v", tt)
                if part == 0:
                    open_chains[key] = prjp.tile([P, FQ], f32, tag="prj", name=f"pv{tt}")
                p = open_chains[key]
                for l in range(4 * part, 4 * part + 4):
                    nc.tensor.matmul(
                        p[:],
                        XT[l][:, tt * P : (tt + 1) * P],
                        WV[l][:],
                        start=(l == 0),
                        stop=(l == KC - 1),
                    )
                if part == 1:
                    del open_chains[key]
                    vdst = V[tt][:].rearrange("p (u c) -> p u c", c=HD + 1)[:, :, 0:HD]
                    vsrc = p[:].rearrange("p (u c) -> p u c", c=HD)
                    nc.vector.tensor_copy(vdst, vsrc)

            def out_chain(tt, h):
                """y[tt-tile, h-half] = AOT-slice^T @ WO, 4 MMs + copy + DMA."""
                p = prjp.tile([P, QC], f32, tag="prj", name=f"py{tt}_{h}")
                for d in range(NDC):
                    nc.tensor.matmul(
                        p[:],
                        AOT[d][:, tt * P : (tt + 1) * P],
                        WO[d][:, h * QC : (h + 1) * QC],
                        start=(d == 0),
                        stop=(d == NDC - 1),
                    )
                ysb = ystp.tile([P, QC], f32, tag="y")
                # alternate eviction engine to balance ACT/DVE load
                if (tt + h) % 2 == 0:
                    nc.scalar.copy(ysb[:], p[:])
                else:
                    nc.vector.tensor_copy(ysb[:], p[:])
                nc.sync.dma_start(
                    out=y[tt * P : (tt + 1) * P, h * QC : (h + 1) * QC],
                    in_=ysb[:],
                )

            # per-hp filler schedules: {step: [unit, ...]}.  hp0's schedule is
            # deadline-driven (chunk c is read by attention from j=c onward at
            # step 4*c*(c+1)/2...; V[tt] is read by PV at the k-step for tile
            # tt of each j >= tt//4).
            sched = {hp: {} for hp in range(4)}

            def put(hp, step, fn):
                sched[hp].setdefault(step, []).append(fn)

            # hp0: QK c1 at steps 0-3 (needed step 4), V4-7 at 4-7 (needed
            # 8-11), QK c2 at 8-9, V8-11 at 10-13 (needed 16+), QK c3 at
            # 14-15, V12-15 at 16-19 (needed 28+)
            s = 0
            for c in (1,):
                for ft in (0, 4):
                    for part in (0, 1):
                        put(0, s, lambda ft=ft, c=c, part=part: qk_half(ft, c, part))
                        s += 1
            for tt in (4, 5, 6, 7):
                for part in (0, 1):
                    put(0, 4 + (tt - 4), lambda tt=tt, part=part: v_half(tt, part))
            s = 8
            for c in (2,):
                for ft in (0, 4):
                    for part in (0, 1):
                        put(0, s, lambda ft=ft, c=c, part=part: qk_half(ft, c, part))
                        s += 1 if part else 0
            for tt in (8, 9, 10, 11):
                for part in (0, 1):
                    put(0, 10 + (tt - 8), lambda tt=tt, part=part: v_half(tt, part))
            s = 14
            for c in (3,):
                for ft in (0, 4):
                    for part in (0, 1):
                        put(0, s, lambda ft=ft, c=c, part=part: qk_half(ft, c, part))
                        s += 1 if part else 0
            for tt in (12, 13, 14, 15):
                for part in (0, 1):
                    put(0, 16 + (tt - 12), lambda tt=tt, part=part: v_half(tt, part))
            # QK(hp+1) spread over each hp's remaining steps
            for hp in range(3):
                base = 20 if hp == 0 else 0
                units = []
                for ft in (hp + 1, 4 + hp + 1):
                    for c in range(NJ):
                        for part in (0, 1):
                            units.append(
                                lambda ft=ft, c=c, part=part: qk_half(ft, c, part)
                            )
                span = 40 - base
                for k, fn in enumerate(units):
                    put(hp, base + (k * span) // len(units), fn)

            # ---------------- pre-attention work ----------------
            for ft in (0, 4):
                for part in (0, 1):
                    qk_half(ft, 0, part)
            for tt in range(4):
                for part in (0, 1):
                    v_half(tt, part)

            # ---------------- attention + interleaved fillers ----------------
            out_ready = []  # out-proj chains unlocked so far
            for hp in range(4):
                step = 0
                for j in range(NJ):
                    nkt = 4 * j + 4
                    pv = {
                        u: pvp.tile([HD + 1, QC], f32, tag="pv", name=f"pv{u}")
                        for u in (0, 1)
                    }
                    for i in range(nkt):
                        s_ = i - 4 * j
                        w0 = 128 * s_ if s_ >= 0 else 0
                        st = bigp.tile([P, 2 * QC], f32, tag="big", name="st")
                        for u in (0, 1):
                            rs = slice(64 * u, 64 * u + 64)
                            nc.tensor.matmul(
                                st[:, u * QC + w0 : (u + 1) * QC],
                                QKT[4 + hp][rs, i * P : (i + 1) * P],
                                QKT[hp][rs, j * QC + w0 : (j + 1) * QC],
                                start=True,
                                stop=True,
                            )
                        win3 = st[:].rearrange("p (h q) -> p h q", h=2)
                        if s_ >= 0:
                            nc.vector.tensor_tensor(
                                out=win3[:, :, w0 : w0 + P],
                                in0=win3[:, :, w0 : w0 + P],
                                in1=cmask[:].rearrange("p (h q) -> p h q", h=2),
                                op=mybir.AluOpType.add,
                            )
                        e = ep.tile([P, 2 * QC], bf16, tag="e", name="e")
                        nc.scalar.activation(
                            e[:].rearrange("p (h q) -> p h q", h=2)[:, :, w0:QC],
                            win3[:, :, w0:QC],
                            mybir.ActivationFunctionType.Exp,
                            scale=0.125,
                        )
                        for u in (0, 1):
                            hloc = 2 * hp + u
                            nc.tensor.matmul(
                                pv[u][:, w0:QC],
                                V[i][:, hloc * (HD + 1) : (hloc + 1) * (HD + 1)],
                                e[:, u * QC + w0 : (u + 1) * QC],
                                start=(i == 0),
                                stop=(i == nkt - 1),
                            )
                        for fn in sched[hp].pop(step, ()):
                            fn()
                        step += 1
                        # out-proj chains become fillers once unlocked
                        if out_ready:
                            out_ready.pop(0)()
                    for u in (0, 1):
                        # normalization: copy accumulator + denominator row out
                        # of PSUM, broadcast the denominator, reciprocal,
                        # multiply into AOT.  (NB: reciprocal_approx_fast is a
                        # custom-DVE op — feeding it a cross-partition PSUM
                        # read produces garbage on HW, keep it SBUF-to-SBUF.)
                        sa = nrmp.tile([HD, QC], f32, tag="sa", name="sa")
                        nc.vector.tensor_copy(sa[:], pv[u][0:HD, :])
                        sd = nrmp.tile([1, QC], f32, tag="sd", name="sd")
                        nc.vector.tensor_copy(sd[:], pv[u][HD : HD + 1, :])
                        bc = nrmp.tile([HD, QC], f32, tag="bc", name="bc")
                        nc.gpsimd.partition_broadcast(bc[:], sd[:])
                        nc.vector.reciprocal_approx_fast(bc[:], bc[:])
                        nc.vector.tensor_tensor(
                            out=AOT[hp][
                                64 * u : 64 * u + 64, j * QC : (j + 1) * QC
                            ],
                            in0=sa[:],
                            in1=bc[:],
                            op=mybir.AluOpType.mult,
                        )
                    # after the last head-pair finishes chunk j, its tokens'
                    # output projection is unlocked
                    if hp == 3:
                        for tt in range(4 * j, 4 * j + 4):
                            for h in (0, 1):
                                out_ready.append(
                                    lambda tt=tt, h=h: out_chain(tt, h)
                                )
                # drain any unconsumed fillers before the next head-pair
                for st_ in sorted(sched[hp]):
                    for fn in sched[hp][st_]:
                        fn()
                sched[hp] = {}
            # drain remaining out-proj chains
            while out_ready:
                out_ready.pop(0)()
            if debug_dumps:
                for ft in range(8):
                    nc.sync.dma_start(
                        out=dbg_qkt[ft * 128 : (ft + 1) * 128, :], in_=QKT[ft][:]
                    )
                for tt in range(2):
                    nc.sync.dma_start(
                        out=dbg_v[tt * 128 : (tt + 1) * 128, :], in_=V[tt][:]
                    )
                for d in range(4):
                    nc.sync.dma_start(
                        out=dbg_aot[d * 128 : (d + 1) * 128, :], in_=AOT[d][:]
                    )
    nc.compile()
    return nc


def get_program(tok=T):
    if tok not in _prog_cache:
        _prog_cache[tok] = build_program(tok)
    return _prog_cache[tok]


def _pack_pmaj(a, nchunk):
    """[nchunk*128, F] -> [128, nchunk*F] partition-major."""
    F = a.shape[1]
    return np.ascontiguousarray(
        a.reshape(nchunk, 128, F).transpose(1, 0, 2).reshape(128, nchunk * F)
    )


def make_in_maps(x, w_qkv, w_out):
    """Shard full inputs into 8 per-core input maps (bf16, packed layouts)."""
    bf = ml_dtypes.bfloat16
    x = np.asarray(x, dtype=np.float32)
    w_qkv = np.asarray(w_qkv, dtype=np.float32).astype(bf)
    w_out = np.asarray(w_out, dtype=np.float32).astype(bf)
    D = D_MODEL
    xTs = [_pack_pmaj(np.ascontiguousarray(x[b].T).astype(bf), 8) for b in range(x.shape[0])]
    in_maps = []
    for c in range(N_CORES):
        b, hg = c // 2, c % 2
        in_maps.append(
            {
                "xT": xTs[b],
                "wq": _pack_pmaj(w_qkv[:, hg * FQ : (hg + 1) * FQ], 8),
                "wk": _pack_pmaj(w_qkv[:, D + hg * FQ : D + (hg + 1) * FQ], 8),
                "wv": _pack_pmaj(w_qkv[:, 2 * D + hg * FQ : 2 * D + (hg + 1) * FQ], 8),
                "wo": _pack_pmaj(w_out[hg * FQ : (hg + 1) * FQ, :], 4),
            }
        )
    return in_maps


_runner_cache = {}


def _make_runner(nc, n_cores=N_CORES):
    """Cached multi-core executor (same semantics as bass2jax.run_bass_via_pjrt
    for a program with no partition-id and no debug tensors, but the jitted
    callable is reusable so repeat kernel() calls don't recompile)."""
    import jax
    from jax.sharding import Mesh, PartitionSpec
    from jax.experimental.shard_map import shard_map
    import concourse.mybir as mybir
    from concourse.bass2jax import _bass_exec_p, install_neuronx_cc_hook

    install_neuronx_cc_hook()

    in_names, out_names, out_avals = [], [], []
    for alloc in nc.m.functions[0].allocations:
        if not isinstance(alloc, mybir.MemoryLocationSet):
            continue
        name = alloc.memorylocations[0].name
        if alloc.kind == "ExternalInput":
            in_names.append(name)
        elif alloc.kind == "ExternalOutput":
            out_names.append(name)
            out_avals.append(
                jax.core.ShapedArray(
                    tuple(alloc.tensor_shape), mybir.dt.np(alloc.dtype)
                )
            )
    n_params = len(in_names)
    n_outs = len(out_avals)
    all_in_names = in_names + out_names

    def _body(*args):
        outs = _bass_exec_p.bind(
            *args,
            out_avals=tuple(out_avals),
            in_names=tuple(all_in_names),
            out_names=tuple(out_names),
            lowering_input_output_aliases=(),
            sim_require_finite=True,
            sim_require_nnan=True,
            nc=nc,
        )
        return tuple(outs)

    devices = jax.devices()[:n_cores]
    mesh = Mesh(np.asarray(devices), ("core",))
    donate = tuple(range(n_params, n_params + n_outs))
    sharded = jax.jit(
        shard_map(
            _body,
            mesh=mesh,
            in_specs=(PartitionSpec("core"),) * (n_params + n_outs),
            out_specs=(PartitionSpec("core"),) * n_outs,
            check_rep=False,
        ),
        donate_argnums=donate,
        keep_unused=True,
    )

    def run(in_maps):
        per_core = [[np.asarray(m[nm]) for nm in in_names] for m in in_maps]
        concat_in = [
            np.concatenate([per_core[c][i] for c in range(n_cores)], axis=0)
            for i in range(n_params)
        ]
        concat_zeros = [
            np.zeros((n_cores * a.shape[0], *a.shape[1:]), a.dtype)
            for a in out_avals
        ]
        out_arrs = sharded(*concat_in, *concat_zeros)
        return [
            {
                nm: np.asarray(out_arrs[i]).reshape(n_cores, *out_avals[i].shape)[c]
                for i, nm in enumerate(out_names)
            }
            for c in range(n_cores)
        ]

    return run


def get_runner(tok=T):
    if tok not in _runner_cache:
        _runner_cache[tok] = _make_runner(get_program(tok))
    return _runner_cache[tok]


def kernel(x, w_qkv, w_out, b_out):
    in_maps = make_in_maps(x, w_qkv, w_out)
    try:
        run = get_runner(T)
        results = run(in_maps)
    except Exception:
        # fallback: the stock SPMD runner (recompiles per call but is the
        # battle-tested path)
        from concourse.bass_utils import run_bass_kernel_spmd

        results = run_bass_kernel_spmd(
            get_program(T), in_maps, list(range(N_CORES))
        ).results
    b_out = np.asarray(b_out, dtype=np.float32)
    out = np.empty((B, T, D_MODEL), dtype=np.float32)
    for b in range(B):
        out[b] = results[2 * b]["y"] + results[2 * b + 1]["y"] + b_out
    return out


# revision 17
# speedup vs baseline: 1.0564x; 1.0248x over previous
"""Causal self-attention TRN2 Bass kernel (bf16, software-pipelined).

Sharding: 8 cores = 4 batches x 2 head-groups. Core c handles batch c//2 and
heads (c%2)*8 .. (c%2)*8+8 (of 16). Each core computes its heads' attention
and a partial output projection; the host sums the two partials per batch and
adds b_out.

Design notes:
  - all matmul operands bf16 (FWL weight loads overlap the stream; f32r
    self-loading matmuls serialize a ~180ns weight load per matmul)
  - all DRAM inputs are host-packed to [128, N] partition-major layouts so
    every DMA moves 8-32KB contiguous rows (descriptor-rate limited otherwise)
  - single pool scope, one long instruction stream: QK/V/out projections are
    emitted as deadline-scheduled "filler" half-chains interleaved into the
    ACT-bound attention cadence, so the PE never idles and HAM stays at 8/8
  - softmax denominator via the ones-column of V (row 64 of the PV PSUM)

Layouts on chip (per core):
  XT    [128, 8x2048] bf16   x[b].T, d-major l-tiles (views XT[l])
  WQ/WK/WV [128, 8x512] bf16, WO [128, 4x1024] bf16
  QKT   8 x [128, 2048] bf16  Q^T (0..3) / K^T (4..7) features x tokens
  V     16 x [128, 520] bf16  tokens x (8 heads x (64 vals + ones col))
  e     [128, 1024] bf16      exp(S^T) per k-tile, both heads
  AOT   4 x [128, 2048] bf16  normalized attention out (features x tokens)
  y     [2048, 1024] f32      partial output projection
"""
import sys

sys.path.insert(0, "/opt/trn_rl_repo")

import numpy as np
import ml_dtypes

D_MODEL = 1024
N_HEADS = 16
B = 4
T = 2048
HD = 64
N_CORES = 8
NH_LOC = N_HEADS // 2  # heads per core
FQ = NH_LOC * HD  # 512 local features

_prog_cache = {}


def build_program(tok=T, debug_dumps=False):
    """Build the single-core SPMD Bass program. tok must be a multiple of 512."""
    import concourse.mybir as mybir
    import concourse.tile as tile
    from concourse import bacc

    f32 = mybir.dt.float32
    bf16 = mybir.dt.bfloat16
    P = 128
    QC = 512  # q-chunk width
    KC = D_MODEL // P  # 8 d-model chunks
    TT = tok // P  # token tiles
    NJ = tok // QC  # q-chunks
    NDC = FQ // P  # 4 feature chunks

    nc = bacc.Bacc("TRN2", target_bir_lowering=False, debug=False, num_devices=N_CORES)

    xT = nc.dram_tensor("xT", [P, KC * tok], bf16, kind="ExternalInput")
    wq = nc.dram_tensor("wq", [P, KC * FQ], bf16, kind="ExternalInput")
    wk = nc.dram_tensor("wk", [P, KC * FQ], bf16, kind="ExternalInput")
    wv = nc.dram_tensor("wv", [P, KC * FQ], bf16, kind="ExternalInput")
    wo = nc.dram_tensor("wo", [P, NDC * D_MODEL], bf16, kind="ExternalInput")
    y = nc.dram_tensor("y", [tok, D_MODEL], f32, kind="ExternalOutput")
    if debug_dumps:
        dbg_qkt = nc.dram_tensor("dbg_qkt", [8 * 128, tok], bf16, kind="ExternalOutput")
        dbg_v = nc.dram_tensor("dbg_v", [2 * 128, NH_LOC * (HD + 1)], bf16, kind="ExternalOutput")
        dbg_aot = nc.dram_tensor("dbg_aot", [4 * 128, tok], bf16, kind="ExternalOutput")

    with tile.TileContext(nc) as tc:
        with (
            tc.tile_pool(name="wqp", bufs=1) as wqp,
            tc.tile_pool(name="wkp", bufs=1) as wkp,
            tc.tile_pool(name="wvp", bufs=1) as wvp,
            tc.tile_pool(name="wop", bufs=1) as wop,
            tc.tile_pool(name="xtp", bufs=1) as xtp,
            tc.tile_pool(name="qktp", bufs=1) as qktp,
            tc.tile_pool(name="vp", bufs=1) as vp,
            tc.tile_pool(name="aotp", bufs=1) as aotp,
            tc.tile_pool(name="ep", bufs=3) as ep,
            tc.tile_pool(name="ystp", bufs=3) as ystp,
            tc.tile_pool(name="mvp", bufs=1) as mvp,
            tc.tile_pool(name="nrm", bufs=4) as nrmp,
            tc.tile_pool(name="big", bufs=2, space="PSUM") as bigp,   # 4 banks
            tc.tile_pool(name="pvp", bufs=2, space="PSUM") as pvp,    # 2 banks
            tc.tile_pool(name="prj", bufs=2, space="PSUM") as prjp,   # 2 banks
        ):
            WQa = wqp.tile([P, KC * FQ], bf16, tag="wq", name="wq")
            WKa = wkp.tile([P, KC * FQ], bf16, tag="wk", name="wk")
            WVa = wvp.tile([P, KC * FQ], bf16, tag="wv", name="wv")
            WOa = wop.tile([P, NDC * D_MODEL], bf16, tag="wo", name="wo")
            XTa = xtp.tile([P, KC * tok], bf16, tag="xt", name="xt")
            WQ = [WQa[:, l * FQ : (l + 1) * FQ] for l in range(KC)]
            WK = [WKa[:, l * FQ : (l + 1) * FQ] for l in range(KC)]
            WV = [WVa[:, l * FQ : (l + 1) * FQ] for l in range(KC)]
            WO = [WOa[:, d * D_MODEL : (d + 1) * D_MODEL] for d in range(NDC)]
            XT = [XTa[:, l * tok : (l + 1) * tok] for l in range(KC)]
            QKT = [qktp.tile([P, tok], bf16, tag=f"qkt{i}", name=f"qkt{i}") for i in range(8)]
            V = [vp.tile([P, NH_LOC * (HD + 1)], bf16, tag=f"v{i}", name=f"v{i}") for i in range(TT)]
            AOT = [aotp.tile([P, tok], bf16, tag=f"aot{d}", name=f"aot{d}") for d in range(NDC)]

            # ---------------- input DMAs ----------------
            # weights on the sync queue, x tiles on the gpsimd queue: the two
            # queues issue in parallel.  Splits sized so the first QK chains
            # can start ~2us in (chains walk l ascending).
            nc.sync.dma_start(out=WQa[:, : 4 * FQ], in_=wq[:, : 4 * FQ])
            nc.gpsimd.dma_start(out=XTa[:, : 2 * tok], in_=xT[:, : 2 * tok])
            nc.sync.dma_start(out=WQa[:, 4 * FQ :], in_=wq[:, 4 * FQ :])
            nc.gpsimd.dma_start(
                out=XTa[:, 2 * tok : 4 * tok], in_=xT[:, 2 * tok : 4 * tok]
            )
            nc.sync.dma_start(out=WKa[:], in_=wk[:])
            nc.gpsimd.dma_start(
                out=XTa[:, 4 * tok : 6 * tok], in_=xT[:, 4 * tok : 6 * tok]
            )
            nc.sync.dma_start(out=WVa[:], in_=wv[:])
            nc.gpsimd.dma_start(out=XTa[:, 6 * tok :], in_=xT[:, 6 * tok :])
            nc.sync.dma_start(out=WOa[:], in_=wo[:])

            # warm the exp table while input DMAs stream
            warm = mvp.tile([1, 8], f32, tag="warm", name="warm")
            nc.gpsimd.memset(warm[:], 0.0)
            nc.scalar.activation(warm[:], warm[:], mybir.ActivationFunctionType.Exp)

            # causal mask triangle, duplicated for the two heads:
            # cmask[p, q] = 0 where q >= p else -1e30, shape [128, 2*128]
            cmask = mvp.tile([P, 2 * P], bf16, tag="cmask", name="cmask")
            nc.gpsimd.memset(cmask[:], 0.0)
            for half in (0, 1):
                nc.gpsimd.affine_select(
                    out=cmask[:, half * P : (half + 1) * P],
                    in_=cmask[:, half * P : (half + 1) * P],
                    compare_op=mybir.AluOpType.is_ge,
                    fill=-1e30,
                    base=0,
                    pattern=[[1, P]],
                    channel_multiplier=-1,
                )
            # ones columns of V: memset whole tiles (value cols overwritten by
            # the projection eviction; col 64 of each head group keeps 1.0)
            for tt in range(TT):
                nc.gpsimd.memset(V[tt][:], 1.0)

            # ---------------- filler chains (emitted in halves) ----------------
            open_chains = {}

            def qk_half(ft, c, part):
                """QKT[ft][:, c-chunk] = (w-slice)^T @ XT over l; 2 halves."""
                wsrc = WQ if ft < 4 else WK
                fo = (ft % 4) * P
                key = ("qk", ft, c)
                if part == 0:
                    open_chains[key] = prjp.tile([P, QC], f32, tag="prj", name=f"pqk{ft}_{c}")
                p = open_chains[key]
                for l in range(4 * part, 4 * part + 4):
                    nc.tensor.matmul(
                        p[:],
                        wsrc[l][:, fo : fo + P],
                        XT[l][:, c * QC : (c + 1) * QC],
                        start=(l == 0),
                        stop=(l == KC - 1),
                    )
                if part == 1:
                    del open_chains[key]
                    nc.vector.tensor_copy(QKT[ft][:, c * QC : (c + 1) * QC], p[:])

            def v_half(tt, part):
                """V[tt] value cols = XT-slice^T @ WV; 2 halves."""
                key = ("v", tt)
                if part == 0:
                    open_chains[key] = prjp.tile([P, FQ], f32, tag="prj", name=f"pv{tt}")
                p = open_chains[key]
                for l in range(4 * part, 4 * part + 4):
                    nc.tensor.matmul(
                        p[:],
                        XT[l][:, tt * P : (tt + 1) * P],
                        WV[l][:],
                        start=(l == 0),
                        stop=(l == KC - 1),
                    )
                if part == 1:
                    del open_chains[key]
                    vdst = V[tt][:].rearrange("p (u c) -> p u c", c=HD + 1)[:, :, 0:HD]
                    vsrc = p[:].rearrange("p (u c) -> p u c", c=HD)
                    nc.vector.tensor_copy(vdst, vsrc)

            def out_chain(tt, h):
                """y[tt-tile, h-half] = AOT-slice^T @ WO, 4 MMs + copy + DMA."""
                p = prjp.tile([P, QC], f32, tag="prj", name=f"py{tt}_{h}")
                for d in range(NDC):
                    nc.tensor.matmul(
                        p[:],
                        AOT[d][:, tt * P : (tt + 1) * P],
                        WO[d][:, h * QC : (h + 1) * QC],
                        start=(d == 0),
                        stop=(d == NDC - 1),
                    )
                ysb = ystp.tile([P, QC], f32, tag="y")
                # alternate eviction engine to balance ACT/DVE load
                if (tt + h) % 2 == 0:
                    nc.scalar.copy(ysb[:], p[:])
                else:
                    nc.vector.tensor_copy(ysb[:], p[:])
                nc.sync.dma_start(
                    out=y[tt * P : (tt + 1) * P, h * QC : (h + 1) * QC],
                    in_=ysb[:],
                )

            # per-hp filler schedules: {step: [unit, ...]}.  hp0's schedule is
            # deadline-driven (chunk c is read by attention from j=c onward at
            # step 4*c*(c+1)/2...; V[tt] is read by PV at the k-step for tile
            # tt of each j >= tt//4).
            sched = {hp: {} for hp in range(4)}

            def put(hp, step, fn):
                sched[hp].setdefault(step, []).append(fn)

            # hp0: QK c1 at steps 0-3 (needed step 4), V4-7 at 4-7 (needed
            # 8-11), QK c2 at 8-9, V8-11 at 10-13 (needed 16+), QK c3 at
            # 14-15, V12-15 at 16-19 (needed 28+)
            s = 0
            for c in (1,):
                for ft in (0, 4):
                    for part in (0, 1):
                        put(0, s, lambda ft=ft, c=c, part=part: qk_half(ft, c, part))
                        s += 1
            for tt in (4, 5, 6, 7):
                for part in (0, 1):
                    put(0, 4 + (tt - 4), lambda tt=tt, part=part: v_half(tt, part))
            s = 8
            for c in (2,):
                for ft in (0, 4):
                    for part in (0, 1):
                        put(0, s, lambda ft=ft, c=c, part=part: qk_half(ft, c, part))
                        s += 1 if part else 0
            for tt in (8, 9, 10, 11):
                for part in (0, 1):
                    put(0, 10 + (tt - 8), lambda tt=tt, part=part: v_half(tt, part))
            s = 14
            for c in (3,):
                for ft in (0, 4):
                    for part in (0, 1):
                        put(0, s, lambda ft=ft, c=c, part=part: qk_half(ft, c, part))
                        s += 1 if part else 0
            for tt in (12, 13, 14, 15):
                for part in (0, 1):
                    put(0, 16 + (tt - 12), lambda tt=tt, part=part: v_half(tt, part))
            # QK(hp+1) spread over each hp's remaining steps
            for hp in range(3):
                base = 20 if hp == 0 else 0
                units = []
                for ft in (hp + 1, 4 + hp + 1):
                    for c in range(NJ):
                        for part in (0, 1):
                            units.append(
                                lambda ft=ft, c=c, part=part: qk_half(ft, c, part)
                            )
                span = 40 - base
                for k, fn in enumerate(units):
                    put(hp, base + (k * span) // len(units), fn)

            # ---------------- pre-attention work ----------------
            for ft in (0, 4):
                for part in (0, 1):
                    qk_half(ft, 0, part)
            for tt in range(4):
                for part in (0, 1):
                    v_half(tt, part)

            # ---------------- attention + interleaved fillers ----------------
            out_ready = []  # out-proj chains unlocked so far
            for hp in range(4):
                step = 0
                for j in range(NJ):
                    nkt = 4 * j + 4
                    pv = {
                        u: pvp.tile([HD + 1, QC], f32, tag="pv", name=f"pv{u}")
                        for u in (0, 1)
                    }
                    etiles = {}

                    def emit_s_exp(i, j=j, etiles=etiles):
                        """S-pair + mask + exp for k-tile i of chunk j."""
                        s_ = i - 4 * j
                        w0 = 128 * s_ if s_ >= 0 else 0
                        st = bigp.tile([P, 2 * QC], f32, tag="big", name="st")
                        for u in (0, 1):
                            rs = slice(64 * u, 64 * u + 64)
                            nc.tensor.matmul(
                                st[:, u * QC + w0 : (u + 1) * QC],
                                QKT[4 + hp][rs, i * P : (i + 1) * P],
                                QKT[hp][rs, j * QC + w0 : (j + 1) * QC],
                                start=True,
                                stop=True,
                            )
                        win3 = st[:].rearrange("p (h q) -> p h q", h=2)
                        if s_ >= 0:
                            nc.vector.tensor_tensor(
                                out=win3[:, :, w0 : w0 + P],
                                in0=win3[:, :, w0 : w0 + P],
                                in1=cmask[:].rearrange("p (h q) -> p h q", h=2),
                                op=mybir.AluOpType.add,
                            )
                        e = ep.tile([P, 2 * QC], bf16, tag="e", name="e")
                        nc.scalar.activation(
                            e[:].rearrange("p (h q) -> p h q", h=2)[:, :, w0:QC],
                            win3[:, :, w0:QC],
                            mybir.ActivationFunctionType.Exp,
                            scale=0.125,
                        )
                        etiles[i] = e

                    emit_s_exp(0)
                    for i in range(nkt):
                        # software pipeline: issue next S/exp before this PV so
                        # the PE's in-order queue never heads-of-line blocks
                        # the ACT stream on PV's wait for exp(i)
                        if i + 1 < nkt:
                            emit_s_exp(i + 1)
                        s_ = i - 4 * j
                        w0 = 128 * s_ if s_ >= 0 else 0
                        e = etiles.pop(i)
                        for u in (0, 1):
                            hloc = 2 * hp + u
                            nc.tensor.matmul(
                                pv[u][:, w0:QC],
                                V[i][:, hloc * (HD + 1) : (hloc + 1) * (HD + 1)],
                                e[:, u * QC + w0 : (u + 1) * QC],
                                start=(i == 0),
                                stop=(i == nkt - 1),
                            )
                        for fn in sched[hp].pop(step, ()):
                            fn()
                        step += 1
                        # out-proj chains become fillers once unlocked
                        if out_ready:
                            out_ready.pop(0)()
                    for u in (0, 1):
                        # normalization: copy accumulator + denominator row out
                        # of PSUM, broadcast the denominator, reciprocal,
                        # multiply into AOT.  (NB: reciprocal_approx_fast is a
                        # custom-DVE op — feeding it a cross-partition PSUM
                        # read produces garbage on HW, keep it SBUF-to-SBUF.)
                        sa = nrmp.tile([HD, QC], f32, tag="sa", name="sa")
                        nc.vector.tensor_copy(sa[:], pv[u][0:HD, :])
                        sd = nrmp.tile([1, QC], f32, tag="sd", name="sd")
                        nc.vector.tensor_copy(sd[:], pv[u][HD : HD + 1, :])
                        bc = nrmp.tile([HD, QC], f32, tag="bc", name="bc")
                        nc.gpsimd.partition_broadcast(bc[:], sd[:])
                        nc.vector.reciprocal_approx_fast(bc[:], bc[:])
                        nc.vector.tensor_tensor(
                            out=AOT[hp][
                                64 * u : 64 * u + 64, j * QC : (j + 1) * QC
                            ],
                            in0=sa[:],
                            in1=bc[:],
                            op=mybir.AluOpType.mult,
                        )
                    # after the last head-pair finishes chunk j, its tokens'
                    # output projection is unlocked
                    if hp == 3:
                        for tt in range(4 * j, 4 * j + 4):
                            for h in (0, 1):
                                out_ready.append(
                                    lambda tt=tt, h=h: out_chain(tt, h)
                                )
                # drain any unconsumed fillers before the next head-pair
                for st_ in sorted(sched[hp]):
                    for fn in sched[hp][st_]:
                        fn()
                sched[hp] = {}
            # drain remaining out-proj chains
            while out_ready:
                out_ready.pop(0)()
            if debug_dumps:
                for ft in range(8):
                    nc.sync.dma_start(
                        out=dbg_qkt[ft * 128 : (ft + 1) * 128, :], in_=QKT[ft][:]
                    )
                for tt in range(2):
                    nc.sync.dma_start(
                        out=dbg_v[tt * 128 : (tt + 1) * 128, :], in_=V[tt][:]
                    )
                for d in range(4):
                    nc.sync.dma_start(
                        out=dbg_aot[d * 128 : (d + 1) * 128, :], in_=AOT[d][:]
                    )
    nc.compile()
    return nc


def get_program(tok=T):
    if tok not in _prog_cache:
        _prog_cache[tok] = build_program(tok)
    return _prog_cache[tok]


def _pack_pmaj(a, nchunk):
    """[nchunk*128, F] -> [128, nchunk*F] partition-major."""
    F = a.shape[1]
    return np.ascontiguousarray(
        a.reshape(nchunk, 128, F).transpose(1, 0, 2).reshape(128, nchunk * F)
    )


def make_in_maps(x, w_qkv, w_out):
    """Shard full inputs into 8 per-core input maps (bf16, packed layouts)."""
    bf = ml_dtypes.bfloat16
    x = np.asarray(x, dtype=np.float32)
    w_qkv = np.asarray(w_qkv, dtype=np.float32).astype(bf)
    w_out = np.asarray(w_out, dtype=np.float32).astype(bf)
    D = D_MODEL
    xTs = [_pack_pmaj(np.ascontiguousarray(x[b].T).astype(bf), 8) for b in range(x.shape[0])]
    in_maps = []
    for c in range(N_CORES):
        b, hg = c // 2, c % 2
        in_maps.append(
            {
                "xT": xTs[b],
                "wq": _pack_pmaj(w_qkv[:, hg * FQ : (hg + 1) * FQ], 8),
                "wk": _pack_pmaj(w_qkv[:, D + hg * FQ : D + (hg + 1) * FQ], 8),
                "wv": _pack_pmaj(w_qkv[:, 2 * D + hg * FQ : 2 * D + (hg + 1) * FQ], 8),
                "wo": _pack_pmaj(w_out[hg * FQ : (hg + 1) * FQ, :], 4),
            }
        )
    return in_maps


_runner_cache = {}


def _make_runner(nc, n_cores=N_CORES):
    """Cached multi-core executor (same semantics as bass2jax.run_bass_via_pjrt
    for a program with no partition-id and no debug tensors, but the jitted
    callable is reusable so repeat kernel() calls don't recompile)."""
    import jax
    from jax.sharding import Mesh, PartitionSpec
    from jax.experimental.shard_map import shard_map
    import concourse.mybir as mybir
    from concourse.bass2jax import _bass_exec_p, install_neuronx_cc_hook

    install_neuronx_cc_hook()

    in_names, out_names, out_avals = [], [], []
    for alloc in nc.m.functions[0].allocations:
        if not isinstance(alloc, mybir.MemoryLocationSet):
            continue
        name = alloc.memorylocations[0].name
        if alloc.kind == "ExternalInput":
            in_names.append(name)
        elif alloc.kind == "ExternalOutput":
            out_names.append(name)
            out_avals.append(
                jax.core.ShapedArray(
                    tuple(alloc.tensor_shape), mybir.dt.np(alloc.dtype)
                )
            )
    n_params = len(in_names)
    n_outs = len(out_avals)
    all_in_names = in_names + out_names

    def _body(*args):
        outs = _bass_exec_p.bind(
            *args,
            out_avals=tuple(out_avals),
            in_names=tuple(all_in_names),
            out_names=tuple(out_names),
            lowering_input_output_aliases=(),
            sim_require_finite=True,
            sim_require_nnan=True,
            nc=nc,
        )
        return tuple(outs)

    devices = jax.devices()[:n_cores]
    mesh = Mesh(np.asarray(devices), ("core",))
    donate = tuple(range(n_params, n_params + n_outs))
    sharded = jax.jit(
        shard_map(
            _body,
            mesh=mesh,
            in_specs=(PartitionSpec("core"),) * (n_params + n_outs),
            out_specs=(PartitionSpec("core"),) * n_outs,
            check_rep=False,
        ),
        donate_argnums=donate,
        keep_unused=True,
    )

    def run(in_maps):
        per_core = [[np.asarray(m[nm]) for nm in in_names] for m in in_maps]
        concat_in = [
            np.concatenate([per_core[c][i] for c in range(n_cores)], axis=0)
            for i in range(n_params)
        ]
        concat_zeros = [
            np.zeros((n_cores * a.shape[0], *a.shape[1:]), a.dtype)
            for a in out_avals
        ]
        out_arrs = sharded(*concat_in, *concat_zeros)
        return [
            {
                nm: np.asarray(out_arrs[i]).reshape(n_cores, *out_avals[i].shape)[c]
                for i, nm in enumerate(out_names)
            }
            for c in range(n_cores)
        ]

    return run


def get_runner(tok=T):
    if tok not in _runner_cache:
        _runner_cache[tok] = _make_runner(get_program(tok))
    return _runner_cache[tok]


def kernel(x, w_qkv, w_out, b_out):
    in_maps = make_in_maps(x, w_qkv, w_out)
    try:
        run = get_runner(T)
        results = run(in_maps)
    except Exception:
        # fallback: the stock SPMD runner (recompiles per call but is the
        # battle-tested path)
        from concourse.bass_utils import run_bass_kernel_spmd

        results = run_bass_kernel_spmd(
            get_program(T), in_maps, list(range(N_CORES))
        ).results
    b_out = np.asarray(b_out, dtype=np.float32)
    out = np.empty((B, T, D_MODEL), dtype=np.float32)
    for b in range(B):
        out[b] = results[2 * b]["y"] + results[2 * b + 1]["y"] + b_out
    return out


# revision 23
# speedup vs baseline: 1.0592x; 1.0027x over previous
"""Causal self-attention TRN2 Bass kernel (bf16, software-pipelined).

Sharding: 8 cores = 4 batches x 2 head-groups. Core c handles batch c//2 and
heads (c%2)*8 .. (c%2)*8+8 (of 16). Each core computes its heads' attention
and a partial output projection; the host sums the two partials per batch and
adds b_out.

Design notes:
  - all matmul operands bf16 (FWL weight loads overlap the stream; f32r
    self-loading matmuls serialize a ~180ns weight load per matmul)
  - all DRAM inputs are host-packed to [128, N] partition-major layouts so
    every DMA moves 8-32KB contiguous rows (descriptor-rate limited otherwise)
  - single pool scope, one long instruction stream: QK/V/out projections are
    emitted as deadline-scheduled "filler" half-chains interleaved into the
    ACT-bound attention cadence, so the PE never idles and HAM stays at 8/8
  - softmax denominator via the ones-column of V (row 64 of the PV PSUM)

Layouts on chip (per core):
  XT    [128, 8x2048] bf16   x[b].T, d-major l-tiles (views XT[l])
  WQ/WK/WV [128, 8x512] bf16, WO [128, 4x1024] bf16
  QKT   8 x [128, 2048] bf16  Q^T (0..3) / K^T (4..7) features x tokens
  V     16 x [128, 520] bf16  tokens x (8 heads x (64 vals + ones col))
  e     [128, 1024] bf16      exp(S^T) per k-tile, both heads
  AOT   4 x [128, 2048] bf16  normalized attention out (features x tokens)
  y     [2048, 1024] f32      partial output projection
"""
import sys

sys.path.insert(0, "/opt/trn_rl_repo")

import numpy as np
import ml_dtypes

D_MODEL = 1024
N_HEADS = 16
B = 4
T = 2048
HD = 64
N_CORES = 8
NH_LOC = N_HEADS // 2  # heads per core
FQ = NH_LOC * HD  # 512 local features

_prog_cache = {}


def build_program(tok=T, debug_dumps=False):
    """Build the single-core SPMD Bass program. tok must be a multiple of 512."""
    import concourse.mybir as mybir
    import concourse.tile as tile
    from concourse import bacc

    f32 = mybir.dt.float32
    bf16 = mybir.dt.bfloat16
    P = 128
    QC = 512  # q-chunk width
    KC = D_MODEL // P  # 8 d-model chunks
    TT = tok // P  # token tiles
    NJ = tok // QC  # q-chunks
    NDC = FQ // P  # 4 feature chunks

    nc = bacc.Bacc("TRN2", target_bir_lowering=False, debug=False, num_devices=N_CORES)

    xT = nc.dram_tensor("xT", [P, KC * tok], bf16, kind="ExternalInput")
    wq = nc.dram_tensor("wq", [P, KC * FQ], bf16, kind="ExternalInput")
    wk = nc.dram_tensor("wk", [P, KC * FQ], bf16, kind="ExternalInput")
    wv = nc.dram_tensor("wv", [P, KC * FQ], bf16, kind="ExternalInput")
    wo = nc.dram_tensor("wo", [P, NDC * D_MODEL], bf16, kind="ExternalInput")
    y = nc.dram_tensor("y", [tok, D_MODEL], f32, kind="ExternalOutput")
    if debug_dumps:
        dbg_qkt = nc.dram_tensor("dbg_qkt", [8 * 128, tok], bf16, kind="ExternalOutput")
        dbg_v = nc.dram_tensor("dbg_v", [2 * 128, NH_LOC * (HD + 1)], bf16, kind="ExternalOutput")
        dbg_aot = nc.dram_tensor("dbg_aot", [4 * 128, tok], bf16, kind="ExternalOutput")

    with tile.TileContext(nc) as tc:
        with (
            tc.tile_pool(name="wqp", bufs=1) as wqp,
            tc.tile_pool(name="wkp", bufs=1) as wkp,
            tc.tile_pool(name="wvp", bufs=1) as wvp,
            tc.tile_pool(name="wop", bufs=1) as wop,
            tc.tile_pool(name="xtp", bufs=1) as xtp,
            tc.tile_pool(name="qktp", bufs=1) as qktp,
            tc.tile_pool(name="vp", bufs=1) as vp,
            tc.tile_pool(name="aotp", bufs=1) as aotp,
            tc.tile_pool(name="ep", bufs=3) as ep,
            tc.tile_pool(name="ystp", bufs=3) as ystp,
            tc.tile_pool(name="mvp", bufs=1) as mvp,
            tc.tile_pool(name="nrm", bufs=4) as nrmp,
            tc.tile_pool(name="big", bufs=2, space="PSUM") as bigp,   # 4 banks
            tc.tile_pool(name="pvp", bufs=2, space="PSUM") as pvp,    # 2 banks
            tc.tile_pool(name="prj", bufs=2, space="PSUM") as prjp,   # 2 banks
        ):
            WQa = wqp.tile([P, KC * FQ], bf16, tag="wq", name="wq")
            WKa = wkp.tile([P, KC * FQ], bf16, tag="wk", name="wk")
            WVa = wvp.tile([P, KC * FQ], bf16, tag="wv", name="wv")
            WOa = wop.tile([P, NDC * D_MODEL], bf16, tag="wo", name="wo")
            XTa = xtp.tile([P, KC * tok], bf16, tag="xt", name="xt")
            WQ = [WQa[:, l * FQ : (l + 1) * FQ] for l in range(KC)]
            WK = [WKa[:, l * FQ : (l + 1) * FQ] for l in range(KC)]
            WV = [WVa[:, l * FQ : (l + 1) * FQ] for l in range(KC)]
            WO = [WOa[:, d * D_MODEL : (d + 1) * D_MODEL] for d in range(NDC)]
            XT = [XTa[:, l * tok : (l + 1) * tok] for l in range(KC)]
            QKT = [qktp.tile([P, tok], bf16, tag=f"qkt{i}", name=f"qkt{i}") for i in range(8)]
            V = [vp.tile([P, NH_LOC * (HD + 1)], bf16, tag=f"v{i}", name=f"v{i}") for i in range(TT)]
            AOT = [aotp.tile([P, tok], bf16, tag=f"aot{d}", name=f"aot{d}") for d in range(NDC)]

            # ---------------- input DMAs ----------------
            # weights on the sync queue, x tiles on the gpsimd queue: the two
            # queues issue in parallel.  Halved pieces ordered so the QK c0
            # chains and V[0..3] chains (the pre-attention work) unblock
            # earliest (chains walk l ascending).
            nc.sync.dma_start(out=WQa[:, : 4 * FQ], in_=wq[:, : 4 * FQ])
            nc.gpsimd.dma_start(out=XTa[:, : 2 * tok], in_=xT[:, : 2 * tok])
            nc.sync.dma_start(out=WKa[:, : 4 * FQ], in_=wk[:, : 4 * FQ])
            nc.gpsimd.dma_start(
                out=XTa[:, 2 * tok : 4 * tok], in_=xT[:, 2 * tok : 4 * tok]
            )
            nc.sync.dma_start(out=WVa[:, : 4 * FQ], in_=wv[:, : 4 * FQ])
            nc.sync.dma_start(out=WQa[:, 4 * FQ :], in_=wq[:, 4 * FQ :])
            nc.gpsimd.dma_start(
                out=XTa[:, 4 * tok : 6 * tok], in_=xT[:, 4 * tok : 6 * tok]
            )
            nc.sync.dma_start(out=WKa[:, 4 * FQ :], in_=wk[:, 4 * FQ :])
            nc.sync.dma_start(out=WVa[:, 4 * FQ :], in_=wv[:, 4 * FQ :])
            nc.gpsimd.dma_start(out=XTa[:, 6 * tok :], in_=xT[:, 6 * tok :])
            nc.sync.dma_start(out=WOa[:], in_=wo[:])

            # warm the exp table while input DMAs stream
            warm = mvp.tile([1, 8], f32, tag="warm", name="warm")
            nc.gpsimd.memset(warm[:], 0.0)
            nc.scalar.activation(warm[:], warm[:], mybir.ActivationFunctionType.Exp)

            # causal mask triangle, duplicated for the two heads:
            # cmask[p, q] = 0 where q >= p else -1e30, shape [128, 2*128]
            cmask = mvp.tile([P, 2 * P], bf16, tag="cmask", name="cmask")
            nc.gpsimd.memset(cmask[:], 0.0)
            for half in (0, 1):
                nc.gpsimd.affine_select(
                    out=cmask[:, half * P : (half + 1) * P],
                    in_=cmask[:, half * P : (half + 1) * P],
                    compare_op=mybir.AluOpType.is_ge,
                    fill=-1e30,
                    base=0,
                    pattern=[[1, P]],
                    channel_multiplier=-1,
                )
            # ones columns of V: memset whole tiles (value cols overwritten by
            # the projection eviction; col 64 of each head group keeps 1.0)
            for tt in range(TT):
                nc.gpsimd.memset(V[tt][:], 1.0)

            # ---------------- filler chains (emitted in halves) ----------------
            open_chains = {}

            def qk_half(ft, c, part):
                """QKT[ft][:, c-chunk] = (w-slice)^T @ XT over l; 2 halves."""
                wsrc = WQ if ft < 4 else WK
                fo = (ft % 4) * P
                key = ("qk", ft, c)
                if part == 0:
                    open_chains[key] = prjp.tile([P, QC], f32, tag="prj", name=f"pqk{ft}_{c}")
                p = open_chains[key]
                for l in range(4 * part, 4 * part + 4):
                    nc.tensor.matmul(
                        p[:],
                        wsrc[l][:, fo : fo + P],
                        XT[l][:, c * QC : (c + 1) * QC],
                        start=(l == 0),
                        stop=(l == KC - 1),
                    )
                if part == 1:
                    del open_chains[key]
                    nc.vector.tensor_copy(QKT[ft][:, c * QC : (c + 1) * QC], p[:])

            def v_half(tt, part):
                """V[tt] value cols = XT-slice^T @ WV; 2 halves."""
                key = ("v", tt)
                if part == 0:
                    open_chains[key] = prjp.tile([P, FQ], f32, tag="prj", name=f"pv{tt}")
                p = open_chains[key]
                for l in range(4 * part, 4 * part + 4):
                    nc.tensor.matmul(
                        p[:],
                        XT[l][:, tt * P : (tt + 1) * P],
                        WV[l][:],
                        start=(l == 0),
                        stop=(l == KC - 1),
                    )
                if part == 1:
                    del open_chains[key]
                    vdst = V[tt][:].rearrange("p (u c) -> p u c", c=HD + 1)[:, :, 0:HD]
                    vsrc = p[:].rearrange("p (u c) -> p u c", c=HD)
                    nc.vector.tensor_copy(vdst, vsrc)

            def out_chain(tt, h, tail=False):
                """y[tt-tile, h-half] = AOT-slice^T @ WO, 4 MMs + copy + DMA."""
                p = prjp.tile([P, QC], f32, tag="prj", name=f"py{tt}_{h}")
                for d in range(NDC):
                    nc.tensor.matmul(
                        p[:],
                        AOT[d][:, tt * P : (tt + 1) * P],
                        WO[d][:, h * QC : (h + 1) * QC],
                        start=(d == 0),
                        stop=(d == NDC - 1),
                    )
                ysb = ystp.tile([P, QC], f32, tag="y")
                # tail chains run after the last exp: ACT is idle there.
                # earlier ones alternate engines to balance ACT/DVE load.
                if tail or (tt + h) % 2 == 0:
                    nc.scalar.copy(ysb[:], p[:])
                else:
                    nc.vector.tensor_copy(ysb[:], p[:])
                nc.sync.dma_start(
                    out=y[tt * P : (tt + 1) * P, h * QC : (h + 1) * QC],
                    in_=ysb[:],
                )

            # per-hp filler schedules: {step: [unit, ...]}.  hp0's schedule is
            # deadline-driven (chunk c is read by attention from j=c onward at
            # step 4*c*(c+1)/2...; V[tt] is read by PV at the k-step for tile
            # tt of each j >= tt//4).
            sched = {hp: {} for hp in range(4)}

            def put(hp, step, fn):
                sched[hp].setdefault(step, []).append(fn)

            # hp0, deadline-driven: QK c1 at steps 0-3 (read from step 4),
            # V4-7 at 4-7 (read 8-11), QK c2 at 8-11 (read 12), V8-11 at
            # 12-15 (read 16-19), QK c3 at 16-19 (read 24), V12-15 at 20-23
            # (read 28-31)
            for k, (ft, c) in enumerate([(0, 1), (4, 1)]):
                for part in (0, 1):
                    put(0, 2 * k + part, lambda ft=ft, c=c, part=part: qk_half(ft, c, part))
            for tt in (4, 5, 6, 7):
                for part in (0, 1):
                    put(0, tt, lambda tt=tt, part=part: v_half(tt, part))
            for k, (ft, c) in enumerate([(0, 2), (4, 2)]):
                for part in (0, 1):
                    put(0, 8 + 2 * k + part, lambda ft=ft, c=c, part=part: qk_half(ft, c, part))
            for tt in (8, 9, 10, 11):
                for part in (0, 1):
                    put(0, 4 + tt, lambda tt=tt, part=part: v_half(tt, part))
            for k, (ft, c) in enumerate([(0, 3), (4, 3)]):
                for part in (0, 1):
                    put(0, 16 + 2 * k + part, lambda ft=ft, c=c, part=part: qk_half(ft, c, part))
            for tt in (12, 13, 14, 15):
                for part in (0, 1):
                    put(0, 8 + tt, lambda tt=tt, part=part: v_half(tt, part))
            # QK(hp+1) spread over each hp's steps (hp0's start after its
            # deadline units; hp1/hp2 evenly — ~1 unit per 2.5 steps keeps
            # the per-step PE load under the exp cadence)
            for hp in range(3):
                base = 24 if hp == 0 else 0
                units = []
                for ft in (hp + 1, 4 + hp + 1):
                    for c in range(NJ):
                        for part in (0, 1):
                            units.append(
                                lambda ft=ft, c=c, part=part: qk_half(ft, c, part)
                            )
                span = 40 - base
                for k, fn in enumerate(units):
                    put(hp, base + (k * span) // len(units), fn)

            # ---------------- pre-attention work ----------------
            for ft in (0, 4):
                for part in (0, 1):
                    qk_half(ft, 0, part)
            for tt in range(4):
                for part in (0, 1):
                    v_half(tt, part)

            # ---------------- attention + interleaved fillers ----------------
            out_ready = []  # out-proj chains unlocked so far
            for hp in range(4):
                step = 0
                for j in range(NJ):
                    nkt = 4 * j + 4
                    pv = {
                        u: pvp.tile([HD + 1, QC], f32, tag="pv", name=f"pv{u}")
                        for u in (0, 1)
                    }
                    etiles = {}

                    def emit_s_exp(i, j=j, etiles=etiles):
                        """S-pair + mask + exp for k-tile i of chunk j."""
                        s_ = i - 4 * j
                        w0 = 128 * s_ if s_ >= 0 else 0
                        st = bigp.tile([P, 2 * QC], f32, tag="big", name="st")
                        for u in (0, 1):
                            rs = slice(64 * u, 64 * u + 64)
                            nc.tensor.matmul(
                                st[:, u * QC + w0 : (u + 1) * QC],
                                QKT[4 + hp][rs, i * P : (i + 1) * P],
                                QKT[hp][rs, j * QC + w0 : (j + 1) * QC],
                                start=True,
                                stop=True,
                            )
                        win3 = st[:].rearrange("p (h q) -> p h q", h=2)
                        if s_ >= 0:
                            nc.vector.tensor_tensor(
                                out=win3[:, :, w0 : w0 + P],
                                in0=win3[:, :, w0 : w0 + P],
                                in1=cmask[:].rearrange("p (h q) -> p h q", h=2),
                                op=mybir.AluOpType.add,
                            )
                        e = ep.tile([P, 2 * QC], bf16, tag="e", name="e")
                        nc.scalar.activation(
                            e[:].rearrange("p (h q) -> p h q", h=2)[:, :, w0:QC],
                            win3[:, :, w0:QC],
                            mybir.ActivationFunctionType.Exp,
                            scale=0.125,
                        )
                        etiles[i] = e

                    emit_s_exp(0)
                    for i in range(nkt):
                        # software pipeline: issue next S/exp before this PV so
                        # the PE's in-order queue never heads-of-line blocks
                        # the ACT stream on PV's wait for exp(i)
                        if i + 1 < nkt:
                            emit_s_exp(i + 1)
                        s_ = i - 4 * j
                        w0 = 128 * s_ if s_ >= 0 else 0
                        e = etiles.pop(i)
                        for u in (0, 1):
                            hloc = 2 * hp + u
                            nc.tensor.matmul(
                                pv[u][:, w0:QC],
                                V[i][:, hloc * (HD + 1) : (hloc + 1) * (HD + 1)],
                                e[:, u * QC + w0 : (u + 1) * QC],
                                start=(i == 0),
                                stop=(i == nkt - 1),
                            )
                        for fn in sched[hp].pop(step, ()):
                            fn()
                        step += 1
                        # out-proj chains become fillers once unlocked
                        if out_ready:
                            out_ready.pop(0)()
                    for u in (0, 1):
                        # normalization: copy accumulator + denominator row out
                        # of PSUM, broadcast the denominator, reciprocal,
                        # multiply into AOT.  (NB: reciprocal_approx_fast is a
                        # custom-DVE op — feeding it a cross-partition PSUM
                        # read produces garbage on HW, keep it SBUF-to-SBUF.)
                        sa = nrmp.tile([HD, QC], f32, tag="sa", name="sa")
                        nc.vector.tensor_copy(sa[:], pv[u][0:HD, :])
                        sd = nrmp.tile([1, QC], f32, tag="sd", name="sd")
                        nc.vector.tensor_copy(sd[:], pv[u][HD : HD + 1, :])
                        bc = nrmp.tile([HD, QC], f32, tag="bc", name="bc")
                        nc.gpsimd.partition_broadcast(bc[:], sd[:])
                        nc.vector.reciprocal_approx_fast(bc[:], bc[:])
                        nc.vector.tensor_tensor(
                            out=AOT[hp][
                                64 * u : 64 * u + 64, j * QC : (j + 1) * QC
                            ],
                            in0=sa[:],
                            in1=bc[:],
                            op=mybir.AluOpType.mult,
                        )
                    # after the last head-pair finishes chunk j, its tokens'
                    # output projection is unlocked
                    if hp == 3:
                        for tt in range(4 * j, 4 * j + 4):
                            for h in (0, 1):
                                out_ready.append(
                                    lambda tail=False, tt=tt, h=h: out_chain(
                                        tt, h, tail
                                    )
                                )
                # drain any unconsumed fillers before the next head-pair
                for st_ in sorted(sched[hp]):
                    for fn in sched[hp][st_]:
                        fn()
                sched[hp] = {}
            # drain remaining out-proj chains (tail: evict on idle ACT)
            while out_ready:
                out_ready.pop(0)(True)
            if debug_dumps:
                for ft in range(8):
                    nc.sync.dma_start(
                        out=dbg_qkt[ft * 128 : (ft + 1) * 128, :], in_=QKT[ft][:]
                    )
                for tt in range(2):
                    nc.sync.dma_start(
                        out=dbg_v[tt * 128 : (tt + 1) * 128, :], in_=V[tt][:]
                    )
                for d in range(4):
                    nc.sync.dma_start(
                        out=dbg_aot[d * 128 : (d + 1) * 128, :], in_=AOT[d][:]
                    )
    nc.compile()
    return nc


def get_program(tok=T):
    if tok not in _prog_cache:
        _prog_cache[tok] = build_program(tok)
    return _prog_cache[tok]


def _pack_pmaj(a, nchunk):
    """[nchunk*128, F] -> [128, nchunk*F] partition-major."""
    F = a.shape[1]
    return np.ascontiguousarray(
        a.reshape(nchunk, 128, F).transpose(1, 0, 2).reshape(128, nchunk * F)
    )


def make_in_maps(x, w_qkv, w_out):
    """Shard full inputs into 8 per-core input maps (bf16, packed layouts)."""
    bf = ml_dtypes.bfloat16
    x = np.asarray(x, dtype=np.float32)
    w_qkv = np.asarray(w_qkv, dtype=np.float32).astype(bf)
    w_out = np.asarray(w_out, dtype=np.float32).astype(bf)
    D = D_MODEL
    xTs = [_pack_pmaj(np.ascontiguousarray(x[b].T).astype(bf), 8) for b in range(x.shape[0])]
    in_maps = []
    for c in range(N_CORES):
        b, hg = c // 2, c % 2
        in_maps.append(
            {
                "xT": xTs[b],
                "wq": _pack_pmaj(w_qkv[:, hg * FQ : (hg + 1) * FQ], 8),
                "wk": _pack_pmaj(w_qkv[:, D + hg * FQ : D + (hg + 1) * FQ], 8),
                "wv": _pack_pmaj(w_qkv[:, 2 * D + hg * FQ : 2 * D + (hg + 1) * FQ], 8),
                "wo": _pack_pmaj(w_out[hg * FQ : (hg + 1) * FQ, :], 4),
            }
        )
    return in_maps


_runner_cache = {}


def _make_runner(nc, n_cores=N_CORES):
    """Cached multi-core executor (same semantics as bass2jax.run_bass_via_pjrt
    for a program with no partition-id and no debug tensors, but the jitted
    callable is reusable so repeat kernel() calls don't recompile)."""
    import jax
    from jax.sharding import Mesh, PartitionSpec
    from jax.experimental.shard_map import shard_map
    import concourse.mybir as mybir
    from concourse.bass2jax import _bass_exec_p, install_neuronx_cc_hook

    install_neuronx_cc_hook()

    in_names, out_names, out_avals = [], [], []
    for alloc in nc.m.functions[0].allocations:
        if not isinstance(alloc, mybir.MemoryLocationSet):
            continue
        name = alloc.memorylocations[0].name
        if alloc.kind == "ExternalInput":
            in_names.append(name)
        elif alloc.kind == "ExternalOutput":
            out_names.append(name)
            out_avals.append(
                jax.core.ShapedArray(
                    tuple(alloc.tensor_shape), mybir.dt.np(alloc.dtype)
                )
            )
    n_params = len(in_names)
    n_outs = len(out_avals)
    all_in_names = in_names + out_names

    def _body(*args):
        outs = _bass_exec_p.bind(
            *args,
            out_avals=tuple(out_avals),
            in_names=tuple(all_in_names),
            out_names=tuple(out_names),
            lowering_input_output_aliases=(),
            sim_require_finite=True,
            sim_require_nnan=True,
            nc=nc,
        )
        return tuple(outs)

    devices = jax.devices()[:n_cores]
    mesh = Mesh(np.asarray(devices), ("core",))
    donate = tuple(range(n_params, n_params + n_outs))
    sharded = jax.jit(
        shard_map(
            _body,
            mesh=mesh,
            in_specs=(PartitionSpec("core"),) * (n_params + n_outs),
            out_specs=(PartitionSpec("core"),) * n_outs,
            check_rep=False,
        ),
        donate_argnums=donate,
        keep_unused=True,
    )

    def run(in_maps):
        per_core = [[np.asarray(m[nm]) for nm in in_names] for m in in_maps]
        concat_in = [
            np.concatenate([per_core[c][i] for c in range(n_cores)], axis=0)
            for i in range(n_params)
        ]
        concat_zeros = [
            np.zeros((n_cores * a.shape[0], *a.shape[1:]), a.dtype)
            for a in out_avals
        ]
        out_arrs = sharded(*concat_in, *concat_zeros)
        return [
            {
                nm: np.asarray(out_arrs[i]).reshape(n_cores, *out_avals[i].shape)[c]
                for i, nm in enumerate(out_names)
            }
            for c in range(n_cores)
        ]

    return run


def get_runner(tok=T):
    if tok not in _runner_cache:
        _runner_cache[tok] = _make_runner(get_program(tok))
    return _runner_cache[tok]


def kernel(x, w_qkv, w_out, b_out):
    in_maps = make_in_maps(x, w_qkv, w_out)
    try:
        run = get_runner(T)
        results = run(in_maps)
    except Exception:
        # fallback: the stock SPMD runner (recompiles per call but is the
        # battle-tested path)
        from concourse.bass_utils import run_bass_kernel_spmd

        results = run_bass_kernel_spmd(
            get_program(T), in_maps, list(range(N_CORES))
        ).results
    b_out = np.asarray(b_out, dtype=np.float32)
    out = np.empty((B, T, D_MODEL), dtype=np.float32)
    for b in range(B):
        out[b] = results[2 * b]["y"] + results[2 * b + 1]["y"] + b_out
    return out
